# revision 2
# baseline (speedup 1.0000x reference)
"""Trainium2 Bass kernel for nn_MiniMHCLM (moe_routing).

Strategy (8 NeuronCores, SPMD, one AllGather):
  - embedding gather happens HOST-side (embed[ids] -> bf16, numerically
    identical to the reference's cast); each core is shipped only its
    512-token slice (1 MB) instead of the 103 MB table.
  - per-token pipeline (RMS+phi coeffs, Sinkhorn, gather/scatter mixing)
    is DATA-PARALLEL: core r processes tokens [512r, 512r+512).
  - each core writes its mixed activations TRANSPOSED (k-major,
    [1024, 512] bf16) to an internal DRAM tile; one AllGather
    (1 MB/rank) replicates all 4096 tokens' x_merge^T to every core.
  - vocab-sharded head matmul: core r holds w_head^T rows for vocab
    [r*6283, (r+1)*6283) (host-sliced bf16, core 7 zero-padded) and
    computes logits for all 4096 tokens x its slice; host concatenates.
  - head matmul in bf16 with fp32 PSUM accumulation; PSUM evacuated by
    alternating ACT/DVE copies to bf16 and DMA'd out on two queues.
  - output is bf16 on device (halves HBM writes + host download); host
    converts to fp32.
"""

import numpy as np

HC, C, TMAX = 4, 256, 8
RMS_EPS, PRE_EPS, SINK_EPS, POST_MULT = 1e-6, 1e-4, 1e-6, 2.0
VOCAB = 50257
B, S = 2, 2048
K = HC * C            # 1024
M = HC * HC + 2 * HC  # 24
NKC = K // 128        # 8 k-chunks
NCORES = 8
NT = B * S            # 4096 tokens
TPC = NT // NCORES    # 512 tokens per core
LC = TPC // 128       # 4 local 128-token chunks
VS = 6283             # vocab rows per core (8*6283 = 50264 >= VOCAB)


class _Cfg:
    def __init__(self):
        self.vs = VS
        self.nt = NT
        # head v-blocks: 12 x 512 + 139 (exact VS, no padded compute)
        self.vws = [512] * (VS // 512) + ([VS % 512] if VS % 512 else [])


REAL = _Cfg()


def _build(cfg: _Cfg):
    from contextlib import ExitStack
    from concourse import bass, bacc, mybir
    import concourse.tile as tile
    from concourse.masks import make_identity

    f32 = mybir.dt.float32
    bf16 = mybir.dt.bfloat16
    AX = mybir.AxisListType
    OP = mybir.AluOpType
    AF = mybir.ActivationFunctionType

    vs = cfg.vs
    vws = cfg.vws
    nv = len(vws)
    voff = [sum(vws[:i]) for i in range(nv)]

    nc = bacc.Bacc(target_bir_lowering=False, num_devices=NCORES)
    xr_p = nc.declare_dram_parameter("xr", [TPC, K], bf16, False)
    wvt_p = nc.declare_dram_parameter("wvt", [K, vs], bf16, False)
    wit_p = nc.declare_dram_parameter("wit", [C, C], bf16, False)
    phi_p = nc.declare_dram_parameter("phi", [K, M], bf16, False)
    b_p = nc.declare_dram_parameter("b", [1, M], f32, False)
    al_p = nc.declare_dram_parameter("al", [1, 3], f32, False)
    out_p = nc.declare_dram_parameter("out", [NT, vs], bf16, True)

    # collective staging: k-major x_merge^T, 1 MB per rank -> 8 MB gathered
    in_cc = nc.dram_tensor("in_cc", [K, TPC], bf16)
    out_cc = nc.dram_tensor("out_cc", [NCORES * K, TPC], bf16,
                            addr_space="Shared")

    with ExitStack() as ctx:
        tc = ctx.enter_context(tile.TileContext(nc))
        const = ctx.enter_context(tc.tile_pool(name="const", bufs=1))
        wtp = ctx.enter_context(tc.tile_pool(name="wtp", bufs=1))
        xbfp = ctx.enter_context(tc.tile_pool(name="xbfp", bufs=1))
        scp = ctx.enter_context(tc.tile_pool(name="scp", bufs=1))
        wkA = ctx.enter_context(tc.tile_pool(name="wkA", bufs=2))
        wkB = ctx.enter_context(tc.tile_pool(name="wkB", bufs=2))
        wkC = ctx.enter_context(tc.tile_pool(name="wkC", bufs=2))
        wk3 = ctx.enter_context(tc.tile_pool(name="wk3", bufs=6))
        rkp = ctx.enter_context(tc.tile_pool(name="rkp", bufs=2))
        outp = ctx.enter_context(tc.tile_pool(name="outp", bufs=6))
        pst = ctx.enter_context(tc.tile_pool(name="pst", bufs=2, space="PSUM"))
        psc = ctx.enter_context(tc.tile_pool(name="psc", bufs=1, space="PSUM"))
        pss = ctx.enter_context(tc.tile_pool(name="pss", bufs=2, space="PSUM"))
        psh = ctx.enter_context(tc.tile_pool(name="psh", bufs=3, space="PSUM"))

        # ---------------- prep ----------------
        ident = const.tile([128, 128], bf16)
        make_identity(nc, ident[:])

        cst = const.tile([128, 2], f32)
        nc.vector.memset(cst[:, 0:1], 0.0)
        nc.vector.memset(cst[:, 1:2], RMS_EPS)
        zero_b = cst[:, 0:1]
        eps_b = cst[:, 1:2]

        phi_sb = const.tile([128, NKC * M], bf16)
        for kc in range(NKC):
            nc.sync.dma_start(out=phi_sb[:, kc * M:(kc + 1) * M],
                              in_=phi_p[kc * 128:(kc + 1) * 128, :])
        b_bc = const.tile([128, M], f32)
        nc.sync.dma_start(out=b_bc[:], in_=b_p[0:1, :].to_broadcast([128, M]))
        al_bc = const.tile([128, 3], f32)
        nc.sync.dma_start(out=al_bc[:], in_=al_p[0:1, :].to_broadcast([128, 3]))

        # w_inner^T (host-pretransposed bf16, k-major): one strided DMA
        w_iT = const.tile([128, 2 * C], bf16)
        nc.sync.dma_start(
            out=w_iT[:].rearrange("p (kc o) -> p kc o", kc=2),
            in_=wit_p[:, :].rearrange("(kc p) o -> p kc o", p=128))

        # this core's token slice: [128, LC, K]
        xr_sb = xbfp.tile([128, LC * K], bf16, tag="xr")
        xr3 = xr_sb[:].rearrange("p (c k) -> p c k", c=LC)
        nc.sync.dma_start(
            out=xr3, in_=xr_p[:, :].rearrange("(c p) k -> p c k", p=128))

        # w_head^T slice: 8 row-band DMAs on the scalar HWDGE queue
        wt_all = wtp.tile([128, NKC * vs], bf16, tag="wt_all")
        wt3 = wt_all[:].rearrange("p (kc v) -> p kc v", kc=NKC)
        for kc in range(NKC):
            nc.scalar.dma_start(out=wt_all[:, kc * vs:(kc + 1) * vs],
                                in_=wvt_p[kc * 128:(kc + 1) * 128, :])

        # ---------------- local pipeline (LC chunks) ----------------
        m_all = scp.tile([128, LC * 16], f32, tag="m_all")
        hpre = scp.tile([128, LC * 4], f32, tag="hpre")
        hpost2 = scp.tile([128, LC * 4], f32, tag="hpost2")
        scl = scp.tile([128, LC], f32, tag="scl")
        sqs = scp.tile([128, LC], f32, tag="sqs")
        lg = scp.tile([128, LC * M], f32, tag="lg")
        # all LC chunks' x_merge^T, k-major: [128, kc, tok(LC*128)]
        xmT_all = scp.tile([128, NKC * TPC], bf16, tag="xmT_all")
        xmT3 = xmT_all[:].rearrange("p (kc t) -> p kc t", kc=NKC)

        def pass_a(c):
            xb = xr3[:, c, :]
            dump = wkA.tile([128, K], bf16, tag="dump")
            nc.scalar.activation(out=dump[:], in_=xb, func=AF.Square,
                                 bias=zero_b, accum_out=sqs[:, c:c + 1])
            ptx = pst.tile([128, 1024], bf16, tag="pst")
            for kc in range(NKC):
                nc.tensor.transpose(
                    out=ptx[:, kc * 128:(kc + 1) * 128],
                    in_=xb[:, kc * 128:(kc + 1) * 128], identity=ident[:])
            xT = wkA.tile([128, 1024], bf16, tag="xT")
            nc.scalar.copy(xT[:], ptx[:])
            pc = psc.tile([128, C], f32, tag="psc")
            for kc in range(NKC):
                nc.tensor.matmul(
                    out=pc[:, :M],
                    lhsT=xT[:, kc * 128:(kc + 1) * 128],
                    rhs=phi_sb[:, kc * M:(kc + 1) * M],
                    start=(kc == 0), stop=(kc == NKC - 1))
            nc.vector.tensor_copy(lg[:, c * M:(c + 1) * M], pc[:, :M])

        def pass_post():
            g_sc = LC
            nc.scalar.activation(out=scl[:, :g_sc], in_=sqs[:, :g_sc],
                                 func=AF.Sqrt, scale=1.0 / K, bias=eps_b)
            nc.vector.reciprocal(scl[:, :g_sc], scl[:, :g_sc])
            lgv = lg[:, :g_sc * M].rearrange("p (c m) -> p c m", m=M)
            for c in range(g_sc):
                nc.vector.tensor_scalar_mul(
                    lg[:, c * M:(c + 1) * M], lg[:, c * M:(c + 1) * M],
                    scl[:, c:c + 1])
            nc.vector.tensor_tensor(
                out=lgv, in0=lgv,
                in1=b_bc[:][:, None, :].to_broadcast([128, g_sc, M]),
                op=OP.add)
            nc.scalar.activation(
                out=hpre[:, :g_sc * 4].rearrange("p (c f) -> p c f", f=4),
                in_=lgv[:, :, 0:4], func=AF.Sigmoid, bias=zero_b,
                scale=al_bc[:, 0:1])
            nc.vector.tensor_scalar_add(hpre[:, :g_sc * 4],
                                        hpre[:, :g_sc * 4], PRE_EPS)
            nc.scalar.activation(
                out=hpost2[:, :g_sc * 4].rearrange("p (c f) -> p c f", f=4),
                in_=lgv[:, :, 4:8], func=AF.Sigmoid, bias=zero_b,
                scale=al_bc[:, 1:2])
            nc.vector.tensor_scalar_mul(hpost2[:, :g_sc * 4],
                                        hpost2[:, :g_sc * 4], POST_MULT)
            nc.scalar.activation(
                out=m_all[:, :g_sc * 16].rearrange("p (c f) -> p c f", f=16),
                in_=lgv[:, :, 8:24], func=AF.Exp, bias=zero_b,
                scale=al_bc[:, 2:3])

            ma = m_all[:, :g_sc * 16]
            mv3 = ma.rearrange("p (a i) -> p a i", i=4)
            mv4 = ma.rearrange("p (c o i) -> p c o i", o=4, i=4)
            mv4t = ma.rearrange("p (c o i) -> p c i o", o=4, i=4)
            for _ in range(TMAX):
                rs = wkB.tile([128, LC * 4], f32, tag="rs")
                rsv = rs[:, :g_sc * 4]
                nc.vector.tensor_reduce(rsv, mv3, axis=AX.X, op=OP.add)
                nc.vector.tensor_scalar_add(rsv, rsv, SINK_EPS)
                nc.vector.reciprocal(rsv, rsv)
                nc.vector.tensor_tensor(
                    out=mv3, in0=mv3,
                    in1=rsv[:, :, None].to_broadcast([128, g_sc * 4, 4]),
                    op=OP.mult)
                cs = wkB.tile([128, LC * 4], f32, tag="cs")
                csv = cs[:, :g_sc * 4]
                nc.vector.tensor_reduce(csv, mv4t, axis=AX.X, op=OP.add)
                nc.vector.tensor_scalar_add(csv, csv, SINK_EPS)
                nc.vector.reciprocal(csv, csv)
                nc.vector.tensor_tensor(
                    out=mv4, in0=mv4,
                    in1=csv.rearrange("p (c i) -> p c i", i=4)
                         [:, :, None, :].to_broadcast([128, g_sc, 4, 4]),
                    op=OP.mult)

        def pass_mix(c):
            xb = xr3[:, c, :]
            xin = wkC.tile([128, C], bf16, tag="xin")
            nc.vector.tensor_scalar_mul(
                xin[:], xb[:, 0:C], hpre[:, c * 4:c * 4 + 1])
            for i in range(1, HC):
                tmp = wk3.tile([128, C], bf16, tag="tmp")
                nc.vector.tensor_scalar_mul(
                    tmp[:], xb[:, i * C:(i + 1) * C],
                    hpre[:, c * 4 + i:c * 4 + i + 1])
                eng = nc.vector if i % 2 else nc.gpsimd
                eng.tensor_add(xin[:], xin[:], tmp[:])
            pti = pss.tile([128, C], bf16, tag="pss")
            for ib in range(2):
                nc.tensor.transpose(
                    out=pti[:, ib * 128:(ib + 1) * 128],
                    in_=xin[:, ib * 128:(ib + 1) * 128], identity=ident[:])
            xiT = wkC.tile([128, C], bf16, tag="xiT")
            nc.scalar.copy(xiT[:], pti[:, :C])
            pf = pss.tile([128, C], f32, tag="pss")
            for ib in range(2):
                nc.tensor.matmul(
                    out=pf[:], lhsT=xiT[:, ib * 128:(ib + 1) * 128],
                    rhs=w_iT[:, ib * C:(ib + 1) * C],
                    start=(ib == 0), stop=(ib == 1))
            fo = wkC.tile([128, C], bf16, tag="fo")
            nc.scalar.copy(fo[:], pf[:])

            xmg = wkC.tile([128, K], bf16, tag="xmg")
            for o in range(HC):
                seg = xmg[:, o * C:(o + 1) * C]
                base = c * 16 + o * 4
                nc.vector.tensor_scalar_mul(
                    seg, xb[:, 0:C], m_all[:, base:base + 1])
                for i in range(1, HC):
                    tmp = wk3.tile([128, C], bf16, tag="tmp")
                    nc.vector.tensor_scalar_mul(
                        tmp[:], xb[:, i * C:(i + 1) * C],
                        m_all[:, base + i:base + i + 1])
                    eng = nc.vector if i % 2 else nc.gpsimd
                    eng.tensor_add(seg, seg, tmp[:])
                tmp = wk3.tile([128, C], bf16, tag="tmp")
                nc.vector.tensor_scalar_mul(
                    tmp[:], fo[:], hpost2[:, c * 4 + o:c * 4 + o + 1])
                nc.gpsimd.tensor_add(seg, seg, tmp[:])

            ptm = pst.tile([128, 1024], bf16, tag="pst")
            for kc in range(NKC):
                nc.tensor.transpose(
                    out=ptm[:, kc * 128:(kc + 1) * 128],
                    in_=xmg[:, kc * 128:(kc + 1) * 128], identity=ident[:])
            nc.scalar.copy(
                xmT3[:, :, c * 128:(c + 1) * 128],
                ptm[:].rearrange("p (kc t) -> p kc t", kc=NKC))

        for c in range(LC):
            pass_a(c)
        pass_post()
        for c in range(LC):
            pass_mix(c)

        # ---------------- all-gather x_merge^T ----------------
        for kc in range(NKC):
            nc.sync.dma_start(out=in_cc[kc * 128:(kc + 1) * 128, :],
                              in_=xmT3[:, kc, :])
        nc.gpsimd.collective_compute(
            "AllGather", OP.bypass,
            replica_groups=[list(range(NCORES))],
            ins=[in_cc[:, :]],
            outs=[out_cc[:, :]])

        # ---------------- head matmul over all ranks ----------------
        def readback(rr):
            rkt = rkp.tile([128, NKC * TPC], bf16, tag="rkt", name=f"rk{rr}")
            nc.scalar.dma_start(
                out=rkt[:].rearrange("p (kc t) -> p kc t", kc=NKC),
                in_=out_cc[rr * K:(rr + 1) * K, :]
                    .rearrange("(kc p) t -> p kc t", p=128))
            return rkt[:].rearrange("p (kc t) -> p kc t", kc=NKC)

        cur = readback(0)
        vcount = 0
        for rr in range(NCORES):
            nxt = readback(rr + 1) if rr + 1 < NCORES else None
            for cl in range(LC):
                t0 = (rr * LC + cl) * 128
                for v in range(nv):
                    w, v0 = vws[v], voff[v]
                    ph = psh.tile([128, 512], f32, tag="psh")
                    for kc in range(NKC):
                        nc.tensor.matmul(
                            out=ph[:, :w],
                            lhsT=cur[:, kc, cl * 128:(cl + 1) * 128],
                            rhs=wt3[:, kc, v0:v0 + w],
                            start=(kc == 0), stop=(kc == NKC - 1))
                    stg = outp.tile([128, 512], bf16, tag="stg")
                    if vcount % 2 == 0:
                        nc.scalar.copy(stg[:, :w], ph[:, :w])
                    else:
                        nc.vector.tensor_copy(stg[:, :w], ph[:, :w])
                    deng = nc.sync if vcount % 2 == 0 else nc.gpsimd
                    deng.dma_start(out=out_p[t0:t0 + 128, v0:v0 + w],
                                   in_=stg[:, :w])
                    vcount += 1
            cur = nxt

    if not nc.is_finalized():
        nc.finalize()
    return nc


_NC_CACHE = {}


def _get_nc(cfg):
    key = (cfg.vs, cfg.nt)
    if key not in _NC_CACHE:
        _NC_CACHE[key] = _build(cfg)
    return _NC_CACHE[key]


def _make_in_maps(cfg, input_ids, embed, w_inner, w_head, phi, b,
                  alpha_pre, alpha_post, alpha_res):
    import ml_dtypes
    bf = ml_dtypes.bfloat16

    ids = np.asarray(input_ids).astype(np.int64).reshape(-1)     # [NT]
    x = np.asarray(embed).astype(bf)[ids]                        # [NT, K]
    phi_np = np.asarray(phi).astype(bf)
    wit = np.ascontiguousarray(np.asarray(w_inner).astype(bf).T)
    b_np = np.asarray(b, dtype=np.float32).reshape(1, M)
    al = np.array([[np.asarray(alpha_pre).reshape(-1)[0],
                    np.asarray(alpha_post).reshape(-1)[0],
                    np.asarray(alpha_res).reshape(-1)[0]]], dtype=np.float32)
    wh_t = np.asarray(w_head).astype(bf).T                       # [K, vocab]

    in_maps = []
    for i in range(NCORES):
        sl = wh_t[:, i * VS:(i + 1) * VS]
        if sl.shape[1] < VS:
            wvt = np.zeros((K, VS), bf)
            wvt[:, :sl.shape[1]] = sl
        else:
            wvt = np.ascontiguousarray(sl)
        xr = np.ascontiguousarray(x[i * TPC:(i + 1) * TPC])
        in_maps.append(dict(xr=xr, wvt=wvt, wit=wit, phi=phi_np,
                            b=b_np, al=al))
    return in_maps


def _run(cfg, in_maps, trace=False):
    from concourse.bass_utils import run_bass_kernel_spmd
    nc = _get_nc(cfg)
    return run_bass_kernel_spmd(nc, in_maps, list(range(NCORES)), trace=trace)


def kernel(input_ids, embed, w_inner, w_head, phi, b,
           alpha_pre, alpha_post, alpha_res):
    cfg = REAL
    in_maps = _make_in_maps(cfg, input_ids, embed, w_inner, w_head, phi, b,
                            alpha_pre, alpha_post, alpha_res)
    res = _run(cfg, in_maps).results
    out = np.concatenate(
        [np.asarray(res[i]["out"]).astype(np.float32) for i in range(NCORES)],
        axis=1)[:, :VOCAB]
    return np.ascontiguousarray(out.reshape(B, S, VOCAB))


# revision 4
# speedup vs baseline: 1.0072x; 1.0072x over previous
"""Trainium2 Bass kernel for nn_MiniMHCLM (moe_routing).

Strategy (8 NeuronCores, SPMD, one AllGather):
  - embedding gather happens HOST-side (embed[ids] -> bf16, numerically
    identical to the reference's cast); each core is shipped only its
    512-token slice (1 MB) instead of the 103 MB table.
  - per-token pipeline (RMS+phi coeffs, Sinkhorn, gather/scatter mixing)
    is DATA-PARALLEL: core r processes tokens [512r, 512r+512).
  - each core writes its mixed activations TRANSPOSED (k-major,
    [1024, 512] bf16) to an internal DRAM tile; one AllGather
    (1 MB/rank) replicates all 4096 tokens' x_merge^T to every core.
  - vocab-sharded head matmul: core r holds w_head^T rows for vocab
    [r*6283, (r+1)*6283) (host-sliced bf16, core 7 zero-padded) and
    computes logits for all 4096 tokens x its slice; host concatenates.
  - head matmul in bf16 with fp32 PSUM accumulation; PSUM evacuated by
    alternating ACT/DVE copies to bf16 and DMA'd out on two queues.
  - output is bf16 on device (halves HBM writes + host download); host
    converts to fp32.
"""

import numpy as np

HC, C, TMAX = 4, 256, 8
RMS_EPS, PRE_EPS, SINK_EPS, POST_MULT = 1e-6, 1e-4, 1e-6, 2.0
VOCAB = 50257
B, S = 2, 2048
K = HC * C            # 1024
M = HC * HC + 2 * HC  # 24
NKC = K // 128        # 8 k-chunks
NCORES = 8
NT = B * S            # 4096 tokens
TPC = NT // NCORES    # 512 tokens per core
LC = TPC // 128       # 4 local 128-token chunks
VS = 6283             # vocab rows per core (8*6283 = 50264 >= VOCAB)


class _Cfg:
    def __init__(self):
        self.vs = VS
        self.nt = NT
        # head v-blocks: 12 x 512 + 139 (exact VS, no padded compute)
        self.vws = [512] * (VS // 512) + ([VS % 512] if VS % 512 else [])


REAL = _Cfg()


def _build(cfg: _Cfg):
    from contextlib import ExitStack
    from concourse import bass, bacc, mybir
    import concourse.tile as tile
    from concourse.masks import make_identity

    f32 = mybir.dt.float32
    bf16 = mybir.dt.bfloat16
    AX = mybir.AxisListType
    OP = mybir.AluOpType
    AF = mybir.ActivationFunctionType

    vs = cfg.vs
    vws = cfg.vws
    nv = len(vws)
    voff = [sum(vws[:i]) for i in range(nv)]

    nc = bacc.Bacc(target_bir_lowering=False, num_devices=NCORES)
    xr_p = nc.declare_dram_parameter("xr", [TPC, K], bf16, False)
    wvt_p = nc.declare_dram_parameter("wvt", [K, vs], bf16, False)
    wit_p = nc.declare_dram_parameter("wit", [C, C], bf16, False)
    phi_p = nc.declare_dram_parameter("phi", [K, M], bf16, False)
    b_p = nc.declare_dram_parameter("b", [1, M], f32, False)
    al_p = nc.declare_dram_parameter("al", [1, 3], f32, False)
    out_p = nc.declare_dram_parameter("out", [NT, vs], bf16, True)

    # collective staging: k-major x_merge^T, 1 MB per rank -> 8 MB gathered
    in_cc = nc.dram_tensor("in_cc", [K, TPC], bf16)
    out_cc = nc.dram_tensor("out_cc", [NCORES * K, TPC], bf16,
                            addr_space="Shared")

    with ExitStack() as ctx:
        tc = ctx.enter_context(tile.TileContext(nc))
        const = ctx.enter_context(tc.tile_pool(name="const", bufs=1))
        wtp = ctx.enter_context(tc.tile_pool(name="wtp", bufs=1))
        xbfp = ctx.enter_context(tc.tile_pool(name="xbfp", bufs=1))
        scp = ctx.enter_context(tc.tile_pool(name="scp", bufs=1))
        wkA = ctx.enter_context(tc.tile_pool(name="wkA", bufs=2))
        wkB = ctx.enter_context(tc.tile_pool(name="wkB", bufs=2))
        wkC = ctx.enter_context(tc.tile_pool(name="wkC", bufs=2))
        wk3 = ctx.enter_context(tc.tile_pool(name="wk3", bufs=6))
        rkp = ctx.enter_context(tc.tile_pool(name="rkp", bufs=2))
        outp = ctx.enter_context(tc.tile_pool(name="outp", bufs=6))
        pst = ctx.enter_context(tc.tile_pool(name="pst", bufs=2, space="PSUM"))
        psc = ctx.enter_context(tc.tile_pool(name="psc", bufs=1, space="PSUM"))
        pss = ctx.enter_context(tc.tile_pool(name="pss", bufs=2, space="PSUM"))
        psh = ctx.enter_context(tc.tile_pool(name="psh", bufs=3, space="PSUM"))

        # ---------------- prep ----------------
        # this core's token slice goes FIRST on the sync ring: everything in
        # the pre-AG pipeline hangs off it, and its DMAHW completion lane
        # must not queue behind bulk loads.
        xr_sb = xbfp.tile([128, LC * K], bf16, tag="xr")
        xr3 = xr_sb[:].rearrange("p (c k) -> p c k", c=LC)
        nc.sync.dma_start(
            out=xr3, in_=xr_p[:, :].rearrange("(c p) k -> p c k", p=128))

        ident = const.tile([128, 128], bf16)
        make_identity(nc, ident[:])

        cst = const.tile([128, 2], f32)
        nc.vector.memset(cst[:, 0:1], 0.0)
        nc.vector.memset(cst[:, 1:2], RMS_EPS)
        zero_b = cst[:, 0:1]
        eps_b = cst[:, 1:2]

        phi_sb = const.tile([128, NKC * M], bf16)
        for kc in range(NKC):
            nc.sync.dma_start(out=phi_sb[:, kc * M:(kc + 1) * M],
                              in_=phi_p[kc * 128:(kc + 1) * 128, :])
        b_bc = const.tile([128, M], f32)
        nc.sync.dma_start(out=b_bc[:], in_=b_p[0:1, :].to_broadcast([128, M]))
        al_bc = const.tile([128, 3], f32)
        nc.sync.dma_start(out=al_bc[:], in_=al_p[0:1, :].to_broadcast([128, 3]))

        # w_inner^T (host-pretransposed bf16, k-major): one strided DMA
        w_iT = const.tile([128, 2 * C], bf16)
        nc.sync.dma_start(
            out=w_iT[:].rearrange("p (kc o) -> p kc o", kc=2),
            in_=wit_p[:, :].rearrange("(kc p) o -> p kc o", p=128))

        # w_head^T slice: 8 row-band DMAs on the SWDGE (gpsimd) queue — its
        # completion sems live on the DMASW lanes, so these 12.9 MB of bulk
        # traffic cannot alias the pipeline-critical DMAHW lanes.
        wt_all = wtp.tile([128, NKC * vs], bf16, tag="wt_all")
        wt3 = wt_all[:].rearrange("p (kc v) -> p kc v", kc=NKC)
        for kc in range(NKC):
            nc.gpsimd.dma_start(out=wt_all[:, kc * vs:(kc + 1) * vs],
                                in_=wvt_p[kc * 128:(kc + 1) * 128, :])

        # ---------------- local pipeline (LC chunks) ----------------
        m_all = scp.tile([128, LC * 16], f32, tag="m_all")
        hpre = scp.tile([128, LC * 4], f32, tag="hpre")
        hpost2 = scp.tile([128, LC * 4], f32, tag="hpost2")
        scl = scp.tile([128, LC], f32, tag="scl")
        sqs = scp.tile([128, LC], f32, tag="sqs")
        lg = scp.tile([128, LC * M], f32, tag="lg")
        # all LC chunks' x_merge^T, k-major: [128, kc, tok(LC*128)]
        xmT_all = scp.tile([128, NKC * TPC], bf16, tag="xmT_all")
        xmT3 = xmT_all[:].rearrange("p (kc t) -> p kc t", kc=NKC)

        def pass_a(c):
            xb = xr3[:, c, :]
            dump = wkA.tile([128, K], bf16, tag="dump")
            nc.scalar.activation(out=dump[:], in_=xb, func=AF.Square,
                                 bias=zero_b, accum_out=sqs[:, c:c + 1])
            ptx = pst.tile([128, 1024], bf16, tag="pst")
            for kc in range(NKC):
                nc.tensor.transpose(
                    out=ptx[:, kc * 128:(kc + 1) * 128],
                    in_=xb[:, kc * 128:(kc + 1) * 128], identity=ident[:])
            xT = wkA.tile([128, 1024], bf16, tag="xT")
            nc.scalar.copy(xT[:], ptx[:])
            pc = psc.tile([128, C], f32, tag="psc")
            for kc in range(NKC):
                nc.tensor.matmul(
                    out=pc[:, :M],
                    lhsT=xT[:, kc * 128:(kc + 1) * 128],
                    rhs=phi_sb[:, kc * M:(kc + 1) * M],
                    start=(kc == 0), stop=(kc == NKC - 1))
            nc.vector.tensor_copy(lg[:, c * M:(c + 1) * M], pc[:, :M])

        def pass_post():
            g_sc = LC
            nc.scalar.activation(out=scl[:, :g_sc], in_=sqs[:, :g_sc],
                                 func=AF.Sqrt, scale=1.0 / K, bias=eps_b)
            nc.vector.reciprocal(scl[:, :g_sc], scl[:, :g_sc])
            lgv = lg[:, :g_sc * M].rearrange("p (c m) -> p c m", m=M)
            for c in range(g_sc):
                nc.vector.tensor_scalar_mul(
                    lg[:, c * M:(c + 1) * M], lg[:, c * M:(c + 1) * M],
                    scl[:, c:c + 1])
            nc.vector.tensor_tensor(
                out=lgv, in0=lgv,
                in1=b_bc[:][:, None, :].to_broadcast([128, g_sc, M]),
                op=OP.add)
            nc.scalar.activation(
                out=hpre[:, :g_sc * 4].rearrange("p (c f) -> p c f", f=4),
                in_=lgv[:, :, 0:4], func=AF.Sigmoid, bias=zero_b,
                scale=al_bc[:, 0:1])
            nc.vector.tensor_scalar_add(hpre[:, :g_sc * 4],
                                        hpre[:, :g_sc * 4], PRE_EPS)
            nc.scalar.activation(
                out=hpost2[:, :g_sc * 4].rearrange("p (c f) -> p c f", f=4),
                in_=lgv[:, :, 4:8], func=AF.Sigmoid, bias=zero_b,
                scale=al_bc[:, 1:2])
            nc.vector.tensor_scalar_mul(hpost2[:, :g_sc * 4],
                                        hpost2[:, :g_sc * 4], POST_MULT)
            nc.scalar.activation(
                out=m_all[:, :g_sc * 16].rearrange("p (c f) -> p c f", f=16),
                in_=lgv[:, :, 8:24], func=AF.Exp, bias=zero_b,
                scale=al_bc[:, 2:3])

            ma = m_all[:, :g_sc * 16]
            mv3 = ma.rearrange("p (a i) -> p a i", i=4)
            mv4 = ma.rearrange("p (c o i) -> p c o i", o=4, i=4)
            mv4t = ma.rearrange("p (c o i) -> p c i o", o=4, i=4)
            for _ in range(TMAX):
                rs = wkB.tile([128, LC * 4], f32, tag="rs")
                rsv = rs[:, :g_sc * 4]
                nc.vector.tensor_reduce(rsv, mv3, axis=AX.X, op=OP.add)
                nc.vector.tensor_scalar_add(rsv, rsv, SINK_EPS)
                nc.vector.reciprocal(rsv, rsv)
                nc.vector.tensor_tensor(
                    out=mv3, in0=mv3,
                    in1=rsv[:, :, None].to_broadcast([128, g_sc * 4, 4]),
                    op=OP.mult)
                cs = wkB.tile([128, LC * 4], f32, tag="cs")
                csv = cs[:, :g_sc * 4]
                nc.vector.tensor_reduce(csv, mv4t, axis=AX.X, op=OP.add)
                nc.vector.tensor_scalar_add(csv, csv, SINK_EPS)
                nc.vector.reciprocal(csv, csv)
                nc.vector.tensor_tensor(
                    out=mv4, in0=mv4,
                    in1=csv.rearrange("p (c i) -> p c i", i=4)
                         [:, :, None, :].to_broadcast([128, g_sc, 4, 4]),
                    op=OP.mult)

        def pass_mix(c):
            xb = xr3[:, c, :]
            xin = wkC.tile([128, C], bf16, tag="xin")
            nc.vector.tensor_scalar_mul(
                xin[:], xb[:, 0:C], hpre[:, c * 4:c * 4 + 1])
            for i in range(1, HC):
                tmp = wk3.tile([128, C], bf16, tag="tmp")
                nc.vector.tensor_scalar_mul(
                    tmp[:], xb[:, i * C:(i + 1) * C],
                    hpre[:, c * 4 + i:c * 4 + i + 1])
                eng = nc.vector if i % 2 else nc.gpsimd
                eng.tensor_add(xin[:], xin[:], tmp[:])
            pti = pss.tile([128, C], bf16, tag="pss")
            for ib in range(2):
                nc.tensor.transpose(
                    out=pti[:, ib * 128:(ib + 1) * 128],
                    in_=xin[:, ib * 128:(ib + 1) * 128], identity=ident[:])
            xiT = wkC.tile([128, C], bf16, tag="xiT")
            nc.scalar.copy(xiT[:], pti[:, :C])
            pf = pss.tile([128, C], f32, tag="pss")
            for ib in range(2):
                nc.tensor.matmul(
                    out=pf[:], lhsT=xiT[:, ib * 128:(ib + 1) * 128],
                    rhs=w_iT[:, ib * C:(ib + 1) * C],
                    start=(ib == 0), stop=(ib == 1))
            fo = wkC.tile([128, C], bf16, tag="fo")
            nc.scalar.copy(fo[:], pf[:])

            xmg = wkC.tile([128, K], bf16, tag="xmg")
            for o in range(HC):
                seg = xmg[:, o * C:(o + 1) * C]
                base = c * 16 + o * 4
                nc.vector.tensor_scalar_mul(
                    seg, xb[:, 0:C], m_all[:, base:base + 1])
                for i in range(1, HC):
                    tmp = wk3.tile([128, C], bf16, tag="tmp")
                    nc.vector.tensor_scalar_mul(
                        tmp[:], xb[:, i * C:(i + 1) * C],
                        m_all[:, base + i:base + i + 1])
                    eng = nc.vector if i % 2 else nc.gpsimd
                    eng.tensor_add(seg, seg, tmp[:])
                tmp = wk3.tile([128, C], bf16, tag="tmp")
                nc.vector.tensor_scalar_mul(
                    tmp[:], fo[:], hpost2[:, c * 4 + o:c * 4 + o + 1])
                nc.gpsimd.tensor_add(seg, seg, tmp[:])

            ptm = pst.tile([128, 1024], bf16, tag="pst")
            for kc in range(NKC):
                nc.tensor.transpose(
                    out=ptm[:, kc * 128:(kc + 1) * 128],
                    in_=xmg[:, kc * 128:(kc + 1) * 128], identity=ident[:])
            nc.scalar.copy(
                xmT3[:, :, c * 128:(c + 1) * 128],
                ptm[:].rearrange("p (kc t) -> p kc t", kc=NKC))

        for c in range(LC):
            pass_a(c)
        pass_post()
        for c in range(LC):
            pass_mix(c)

        # ---------------- all-gather x_merge^T ----------------
        for kc in range(NKC):
            nc.sync.dma_start(out=in_cc[kc * 128:(kc + 1) * 128, :],
                              in_=xmT3[:, kc, :])
        nc.gpsimd.collective_compute(
            "AllGather", OP.bypass,
            replica_groups=[list(range(NCORES))],
            ins=[in_cc[:, :]],
            outs=[out_cc[:, :]])

        # ---------------- head matmul over all ranks ----------------
        def readback(rr):
            rkt = rkp.tile([128, NKC * TPC], bf16, tag="rkt", name=f"rk{rr}")
            nc.scalar.dma_start(
                out=rkt[:].rearrange("p (kc t) -> p kc t", kc=NKC),
                in_=out_cc[rr * K:(rr + 1) * K, :]
                    .rearrange("(kc p) t -> p kc t", p=128))
            return rkt[:].rearrange("p (kc t) -> p kc t", kc=NKC)

        cur = readback(0)
        vcount = 0
        for rr in range(NCORES):
            nxt = readback(rr + 1) if rr + 1 < NCORES else None
            for cl in range(LC):
                t0 = (rr * LC + cl) * 128
                for v in range(nv):
                    w, v0 = vws[v], voff[v]
                    ph = psh.tile([128, 512], f32, tag="psh")
                    for kc in range(NKC):
                        nc.tensor.matmul(
                            out=ph[:, :w],
                            lhsT=cur[:, kc, cl * 128:(cl + 1) * 128],
                            rhs=wt3[:, kc, v0:v0 + w],
                            start=(kc == 0), stop=(kc == NKC - 1))
                    stg = outp.tile([128, 512], bf16, tag="stg")
                    if vcount % 2 == 0:
                        nc.scalar.copy(stg[:, :w], ph[:, :w])
                    else:
                        nc.vector.tensor_copy(stg[:, :w], ph[:, :w])
                    deng = nc.sync if vcount % 2 == 0 else nc.gpsimd
                    deng.dma_start(out=out_p[t0:t0 + 128, v0:v0 + w],
                                   in_=stg[:, :w])
                    vcount += 1
            cur = nxt

    if not nc.is_finalized():
        nc.finalize()
    return nc


_NC_CACHE = {}


def _get_nc(cfg):
    key = (cfg.vs, cfg.nt)
    if key not in _NC_CACHE:
        _NC_CACHE[key] = _build(cfg)
    return _NC_CACHE[key]


def _make_in_maps(cfg, input_ids, embed, w_inner, w_head, phi, b,
                  alpha_pre, alpha_post, alpha_res):
    import ml_dtypes
    bf = ml_dtypes.bfloat16

    ids = np.asarray(input_ids).astype(np.int64).reshape(-1)     # [NT]
    x = np.asarray(embed).astype(bf)[ids]                        # [NT, K]
    phi_np = np.asarray(phi).astype(bf)
    wit = np.ascontiguousarray(np.asarray(w_inner).astype(bf).T)
    b_np = np.asarray(b, dtype=np.float32).reshape(1, M)
    al = np.array([[np.asarray(alpha_pre).reshape(-1)[0],
                    np.asarray(alpha_post).reshape(-1)[0],
                    np.asarray(alpha_res).reshape(-1)[0]]], dtype=np.float32)
    wh_t = np.asarray(w_head).astype(bf).T                       # [K, vocab]

    in_maps = []
    for i in range(NCORES):
        sl = wh_t[:, i * VS:(i + 1) * VS]
        if sl.shape[1] < VS:
            wvt = np.zeros((K, VS), bf)
            wvt[:, :sl.shape[1]] = sl
        else:
            wvt = np.ascontiguousarray(sl)
        xr = np.ascontiguousarray(x[i * TPC:(i + 1) * TPC])
        in_maps.append(dict(xr=xr, wvt=wvt, wit=wit, phi=phi_np,
                            b=b_np, al=al))
    return in_maps


def _run(cfg, in_maps, trace=False):
    from concourse.bass_utils import run_bass_kernel_spmd
    nc = _get_nc(cfg)
    return run_bass_kernel_spmd(nc, in_maps, list(range(NCORES)), trace=trace)


def kernel(input_ids, embed, w_inner, w_head, phi, b,
           alpha_pre, alpha_post, alpha_res):
    cfg = REAL
    in_maps = _make_in_maps(cfg, input_ids, embed, w_inner, w_head, phi, b,
                            alpha_pre, alpha_post, alpha_res)
    res = _run(cfg, in_maps).results
    out = np.concatenate(
        [np.asarray(res[i]["out"]).astype(np.float32) for i in range(NCORES)],
        axis=1)[:, :VOCAB]
    return np.ascontiguousarray(out.reshape(B, S, VOCAB))


# revision 9
# speedup vs baseline: 1.0399x; 1.0324x over previous
"""Trainium2 Bass kernel for nn_MiniMHCLM (moe_routing).

Strategy (8 NeuronCores, SPMD, split AllGather):
  - embedding gather happens HOST-side (embed[ids] -> bf16, numerically
    identical to the reference's cast); each core is shipped only its
    512-token slice (1 MB) instead of the 103 MB table.
  - per-token pipeline (RMS+phi coeffs, Sinkhorn, gather/scatter mixing)
    is DATA-PARALLEL: core r processes tokens [512r, 512r+512).
  - a tiny AllReduce barrier early in the kernel absorbs cross-core
    launch skew while const/weight loads + the local pipeline run, so
    the mid-kernel AllGather rendezvous is cheap.
  - mixed activations go out TRANSPOSED (k-major bf16) via TWO
    AllGathers (one per 256-token half) so the second one overlaps the
    head matmul on the first half.
  - vocab-sharded head matmul: core r holds w_head^T for vocab
    [r*6283, (r+1)*6283) (host-sliced bf16, core 7 zero-padded) and
    computes logits for all 4096 tokens x its slice; host concatenates.
  - the shared inner linear is computed per-head (g_i = x_i @ w_inner^T)
    straight from the already-transposed x tiles, f_out = sum_i
    h_pre[i] * g_i -- no extra transposes on the mix critical path.
  - head matmul bf16 with fp32 PSUM accumulation; PSUM evacuated by
    alternating ACT/DVE copies to bf16, DMA'd out on two queues.
  - output is bf16 on device (halves HBM writes + host download); host
    converts to fp32.
"""

import numpy as np

HC, C, TMAX = 4, 256, 8
RMS_EPS, PRE_EPS, SINK_EPS, POST_MULT = 1e-6, 1e-4, 1e-6, 2.0
VOCAB = 50257
B, S = 2, 2048
K = HC * C            # 1024
M = HC * HC + 2 * HC  # 24
NKC = K // 128        # 8 k-chunks
NCORES = 8
NT = B * S            # 4096 tokens
TPC = NT // NCORES    # 512 tokens per core
LC = TPC // 128       # 4 local 128-token chunks
HT = TPC // 2         # 256 tokens per AG half
VS = 6283             # vocab rows per core (8*6283 = 50264 >= VOCAB)


class _Cfg:
    def __init__(self):
        self.vs = VS
        self.nt = NT
        # head v-blocks: 12 x 512 + 139 (exact VS, no padded compute)
        self.vws = [512] * (VS // 512) + ([VS % 512] if VS % 512 else [])


REAL = _Cfg()


def _build(cfg: _Cfg):
    from contextlib import ExitStack
    from concourse import bass, bacc, mybir
    import concourse.tile as tile
    from concourse.masks import make_identity

    f32 = mybir.dt.float32
    bf16 = mybir.dt.bfloat16
    AX = mybir.AxisListType
    OP = mybir.AluOpType
    AF = mybir.ActivationFunctionType

    vs = cfg.vs
    vws = cfg.vws
    nv = len(vws)
    voff = [sum(vws[:i]) for i in range(nv)]

    nc = bacc.Bacc(target_bir_lowering=False, num_devices=NCORES)
    xr_p = nc.declare_dram_parameter("xr", [TPC, K], bf16, False)
    wvt_p = nc.declare_dram_parameter("wvt", [K, vs], bf16, False)
    wit_p = nc.declare_dram_parameter("wit", [C, C], bf16, False)
    phi_p = nc.declare_dram_parameter("phi", [K, M], bf16, False)
    b_p = nc.declare_dram_parameter("b", [1, M], f32, False)
    al_p = nc.declare_dram_parameter("al", [1, 3], f32, False)
    out_p = nc.declare_dram_parameter("out", [NT, vs], bf16, True)

    # collective staging: k-major x_merge^T per 256-token half
    in_cc = [nc.dram_tensor(f"in_cc{h}", [K, HT], bf16) for h in range(2)]
    out_cc = [nc.dram_tensor(f"out_cc{h}", [NCORES * K, HT], bf16,
                             addr_space="Shared") for h in range(2)]
    bar_in = nc.dram_tensor("bar_in", [128, 4], f32)
    bar_out = nc.dram_tensor("bar_out", [128, 4], f32)

    rg = [list(range(NCORES))]

    with ExitStack() as ctx:
        tc = ctx.enter_context(tile.TileContext(nc))
        const = ctx.enter_context(tc.tile_pool(name="const", bufs=1))
        wtp = ctx.enter_context(tc.tile_pool(name="wtp", bufs=1))
        xbfp = ctx.enter_context(tc.tile_pool(name="xbfp", bufs=1))
        scp = ctx.enter_context(tc.tile_pool(name="scp", bufs=1))
        wkA = ctx.enter_context(tc.tile_pool(name="wkA", bufs=2))
        wkB = ctx.enter_context(tc.tile_pool(name="wkB", bufs=2))
        wkC = ctx.enter_context(tc.tile_pool(name="wkC", bufs=2))
        rkp = ctx.enter_context(tc.tile_pool(name="rkp", bufs=3))
        outp = ctx.enter_context(tc.tile_pool(name="outp", bufs=6))
        pst = ctx.enter_context(tc.tile_pool(name="pst", bufs=2, space="PSUM"))
        psc = ctx.enter_context(tc.tile_pool(name="psc", bufs=1, space="PSUM"))
        psg = ctx.enter_context(tc.tile_pool(name="psg", bufs=1, space="PSUM"))
        psh = ctx.enter_context(tc.tile_pool(name="psh", bufs=3, space="PSUM"))

        # ---------------- prep ----------------
        # this core's token slice goes FIRST on the sync ring: everything in
        # the pre-AG pipeline hangs off it, and its DMAHW completion lane
        # must not queue behind bulk loads.
        xr_sb = xbfp.tile([128, LC * K], bf16, tag="xr")
        xr3 = xr_sb[:].rearrange("p (c k) -> p c k", c=LC)
        nc.sync.dma_start(
            out=xr3, in_=xr_p[:, :].rearrange("(c p) k -> p c k", p=128))

        ident = const.tile([128, 128], bf16)
        make_identity(nc, ident[:])

        cst = const.tile([128, 3], f32)
        nc.vector.memset(cst[:, 0:1], 0.0)
        nc.vector.memset(cst[:, 1:2], RMS_EPS)
        nc.vector.memset(cst[:, 2:3], SINK_EPS)
        zero_b = cst[:, 0:1]
        eps_b = cst[:, 1:2]
        sink_b = cst[:, 2:3]

        phi_sb = const.tile([128, NKC * M], bf16)
        for kc in range(NKC):
            nc.sync.dma_start(out=phi_sb[:, kc * M:(kc + 1) * M],
                              in_=phi_p[kc * 128:(kc + 1) * 128, :])
        b_bc = const.tile([128, M], f32)
        nc.sync.dma_start(out=b_bc[:], in_=b_p[0:1, :].to_broadcast([128, M]))
        al_bc = const.tile([128, 3], f32)
        nc.sync.dma_start(out=al_bc[:], in_=al_p[0:1, :].to_broadcast([128, 3]))

        # w_inner^T (host-pretransposed bf16, k-major): one strided DMA
        w_iT = const.tile([128, 2 * C], bf16)
        nc.sync.dma_start(
            out=w_iT[:].rearrange("p (kc o) -> p kc o", kc=2),
            in_=wit_p[:, :].rearrange("(kc p) o -> p kc o", p=128))

        # w_head^T slice: 8 row-band DMAs on the SWDGE (gpsimd) queue — its
        # completion sems live on the DMASW lanes, so these 12.9 MB of bulk
        # traffic cannot alias the pipeline-critical DMAHW lanes.
        wt_all = wtp.tile([128, NKC * vs], bf16, tag="wt_all")
        wt3 = wt_all[:].rearrange("p (kc v) -> p kc v", kc=NKC)
        for kc in range(NKC):
            nc.gpsimd.dma_start(out=wt_all[:, kc * vs:(kc + 1) * vs],
                                in_=wvt_p[kc * 128:(kc + 1) * 128, :])

        # skew-absorbing barrier: a tiny AllReduce on the gpsimd stream.
        # Launch skew between the 8 PJRT executions is eaten HERE, while the
        # other engines keep loading consts and running the local pipeline,
        # instead of at the mid-kernel AllGather rendezvous.
        barz = const.tile([128, 4], f32)
        nc.gpsimd.memset(barz[:], 0.0)
        nc.gpsimd.dma_start(out=bar_in[:, :], in_=barz[:])
        nc.gpsimd.collective_compute(
            "AllReduce", OP.add, replica_groups=rg,
            ins=[bar_in[:, :]], outs=[bar_out[:, :]])

        # ---------------- local pipeline (LC chunks) ----------------
        # m coefficients stored (i outer, o inner) so the mix stage can
        # broadcast a contiguous [128,4] o-column per source head i.
        m_all = scp.tile([128, LC * 16], f32, tag="m_all")
        hpre = scp.tile([128, LC * 4], f32, tag="hpre")
        hpost2 = scp.tile([128, LC * 4], f32, tag="hpost2")
        scl = scp.tile([128, LC], f32, tag="scl")
        sqs = scp.tile([128, LC], f32, tag="sqs")
        lg = scp.tile([128, LC * M], f32, tag="lg")
        # all LC chunks' x_merge^T, k-major: [128, kc, tok(LC*128)]
        xmT_all = scp.tile([128, NKC * TPC], bf16, tag="xmT_all")
        xmT3 = xmT_all[:].rearrange("p (kc t) -> p kc t", kc=NKC)

        g_sbs = {}

        def pass_a(c):
            xb = xr3[:, c, :]
            dump = wkA.tile([128, K], bf16, tag="dump")
            nc.scalar.activation(out=dump[:], in_=xb, func=AF.Square,
                                 bias=zero_b, accum_out=sqs[:, c:c + 1])
            ptx = pst.tile([128, 1024], bf16, tag="pst")
            for kc in range(NKC):
                nc.tensor.transpose(
                    out=ptx[:, kc * 128:(kc + 1) * 128],
                    in_=xb[:, kc * 128:(kc + 1) * 128], identity=ident[:])
            xT = wkA.tile([128, 1024], bf16, tag="xT")
            nc.vector.tensor_copy(xT[:], ptx[:])
            pc = psc.tile([128, C], f32, tag="psc")
            for kc in range(NKC):
                nc.tensor.matmul(
                    out=pc[:, :M],
                    lhsT=xT[:, kc * 128:(kc + 1) * 128],
                    rhs=phi_sb[:, kc * M:(kc + 1) * M],
                    start=(kc == 0), stop=(kc == NKC - 1))
            nc.vector.tensor_copy(lg[:, c * M:(c + 1) * M], pc[:, :M])
            # per-head inner linear: g[t, i*C+o] = sum_k x_i[t,k] w_inner[o,k]
            pg = psg.tile([128, K], f32, tag="psg")
            for kc in range(NKC):
                i = kc // 2
                nc.tensor.matmul(
                    out=pg[:, i * C:(i + 1) * C],
                    lhsT=xT[:, kc * 128:(kc + 1) * 128],
                    rhs=w_iT[:, (kc % 2) * C:(kc % 2 + 1) * C],
                    start=(kc % 2 == 0), stop=(kc % 2 == 1))
            g_sb = wkA.tile([128, K], bf16, tag="g", name=f"g{c}")
            nc.scalar.copy(g_sb[:], pg[:])
            g_sbs[c] = g_sb

        def pass_post():
            g_sc = LC
            nc.scalar.activation(out=scl[:, :g_sc], in_=sqs[:, :g_sc],
                                 func=AF.Sqrt, scale=1.0 / K, bias=eps_b)
            nc.vector.reciprocal(scl[:, :g_sc], scl[:, :g_sc])
            lgv = lg[:, :g_sc * M].rearrange("p (c m) -> p c m", m=M)
            for c in range(g_sc):
                nc.vector.tensor_scalar_mul(
                    lg[:, c * M:(c + 1) * M], lg[:, c * M:(c + 1) * M],
                    scl[:, c:c + 1])
            nc.vector.tensor_tensor(
                out=lgv, in0=lgv,
                in1=b_bc[:][:, None, :].to_broadcast([128, g_sc, M]),
                op=OP.add)
            nc.scalar.activation(
                out=hpre[:, :g_sc * 4].rearrange("p (c f) -> p c f", f=4),
                in_=lgv[:, :, 0:4], func=AF.Sigmoid, bias=zero_b,
                scale=al_bc[:, 0:1])
            nc.vector.tensor_scalar_add(hpre[:, :g_sc * 4],
                                        hpre[:, :g_sc * 4], PRE_EPS)
            nc.scalar.activation(
                out=hpost2[:, :g_sc * 4].rearrange("p (c f) -> p c f", f=4),
                in_=lgv[:, :, 4:8], func=AF.Sigmoid, bias=zero_b,
                scale=al_bc[:, 1:2])
            nc.vector.tensor_scalar_mul(hpost2[:, :g_sc * 4],
                                        hpost2[:, :g_sc * 4], POST_MULT)
            # exp(a_res * res): reference res is [o][i]; store transposed
            # (i outer, o inner) via a strided output view.
            m4t = m_all[:, :g_sc * 16].rearrange(
                "p (c i o) -> p c o i", i=4, o=4)
            nc.scalar.activation(
                out=m4t,
                in_=lgv[:, :, 8:24].rearrange("p c (o i) -> p c o i", o=4),
                func=AF.Exp, bias=zero_b, scale=al_bc[:, 2:3])

            # batched Sinkhorn on (i outer, o inner) storage
            ma = m_all[:, :g_sc * 16]
            mio = ma.rearrange("p (c i o) -> p c i o", i=4, o=4)
            moi = ma.rearrange("p (c i o) -> p c o i", i=4, o=4)
            for _ in range(TMAX):
                # row normalize: per (c, o) sum over i
                rs = wkB.tile([128, LC * 4], f32, tag="rs")
                rs3 = rs[:, :g_sc * 4].rearrange("p (c o) -> p c o", o=4)
                nc.vector.tensor_reduce(rs3, moi, axis=AX.X, op=OP.add)
                nc.vector.tensor_scalar_add(rs[:, :g_sc * 4],
                                            rs[:, :g_sc * 4], SINK_EPS)
                nc.vector.reciprocal(rs[:, :g_sc * 4], rs[:, :g_sc * 4])
                nc.vector.tensor_tensor(
                    out=moi, in0=moi,
                    in1=rs3[:, :, :, None].to_broadcast([128, g_sc, 4, 4]),
                    op=OP.mult)
                # col normalize: per (c, i) sum over o
                cs = wkB.tile([128, LC * 4], f32, tag="cs")
                cs3 = cs[:, :g_sc * 4].rearrange("p (c i) -> p c i", i=4)
                nc.vector.tensor_reduce(cs3, mio, axis=AX.X, op=OP.add)
                nc.vector.tensor_scalar_add(cs[:, :g_sc * 4],
                                            cs[:, :g_sc * 4], SINK_EPS)
                nc.vector.reciprocal(cs[:, :g_sc * 4], cs[:, :g_sc * 4])
                nc.vector.tensor_tensor(
                    out=mio, in0=mio,
                    in1=cs3[:, :, :, None].to_broadcast([128, g_sc, 4, 4]),
                    op=OP.mult)

        def pass_mix(c):
            xb = xr3[:, c, :]
            # f_out = sum_i h_pre[i] * g_i  (g precomputed in pass_a)
            g_sb = g_sbs.pop(c)
            prod = wkC.tile([128, K], bf16, tag="prod")
            nc.vector.tensor_tensor(
                out=prod[:].rearrange("p (i o) -> p i o", i=4),
                in0=g_sb[:].rearrange("p (i o) -> p i o", i=4),
                in1=hpre[:, c * 4:(c + 1) * 4][:, :, None]
                    .to_broadcast([128, 4, C]),
                op=OP.mult)
            fo = wkC.tile([128, C], f32, tag="fo")
            nc.vector.tensor_reduce(
                fo[:], prod[:].rearrange("p (i o) -> p o i", i=4),
                axis=AX.X, op=OP.add)

            # x_merge[o*C+c2] = sum_i m[o,i] x_i[c2] + h_post2[o] fo[c2]
            xmg = wkC.tile([128, K], bf16, tag="xmg")
            xmg4 = xmg[:].rearrange("p (o c2) -> p o c2", o=4)
            tmp4 = wkC.tile([128, K], bf16, tag="tmp4")
            tmp4v = tmp4[:].rearrange("p (o c2) -> p o c2", o=4)
            for i in range(HC):
                mi = m_all[:, c * 16 + i * 4:c * 16 + (i + 1) * 4]
                dst = xmg4 if i == 0 else tmp4v
                nc.vector.tensor_tensor(
                    out=dst,
                    in0=xb[:, i * C:(i + 1) * C][:, None, :]
                        .to_broadcast([128, 4, C]),
                    in1=mi[:, :, None].to_broadcast([128, 4, C]),
                    op=OP.mult)
                if i > 0:
                    nc.vector.tensor_tensor(out=xmg4, in0=xmg4, in1=tmp4v,
                                            op=OP.add)
            nc.vector.tensor_tensor(
                out=tmp4v,
                in0=fo[:][:, None, :].to_broadcast([128, 4, C]),
                in1=hpost2[:, c * 4:(c + 1) * 4][:, :, None]
                    .to_broadcast([128, 4, C]),
                op=OP.mult)
            nc.vector.tensor_tensor(out=xmg4, in0=xmg4, in1=tmp4v, op=OP.add)

            ptm = pst.tile([128, 1024], bf16, tag="pst")
            for kc in range(NKC):
                nc.tensor.transpose(
                    out=ptm[:, kc * 128:(kc + 1) * 128],
                    in_=xmg[:, kc * 128:(kc + 1) * 128], identity=ident[:])
            nc.scalar.copy(
                xmT3[:, :, c * 128:(c + 1) * 128],
                ptm[:].rearrange("p (kc t) -> p kc t", kc=NKC))

        def send_half(h):
            for kc in range(NKC):
                nc.sync.dma_start(
                    out=in_cc[h][kc * 128:(kc + 1) * 128, :],
                    in_=xmT3[:, kc, h * HT:(h + 1) * HT])
            nc.gpsimd.collective_compute(
                "AllGather", OP.bypass, replica_groups=rg,
                ins=[in_cc[h][:, :]], outs=[out_cc[h][:, :]])

        for c in range(LC):
            pass_a(c)
        pass_post()
        pass_mix(0)
        pass_mix(1)
        send_half(0)
        pass_mix(2)
        pass_mix(3)
        send_half(1)

        # ---------------- head matmul over all ranks ----------------
        def readback(h, rr):
            rkt = rkp.tile([128, NKC * HT], bf16, tag="rkt",
                           name=f"rk{h}_{rr}")
            nc.scalar.dma_start(
                out=rkt[:].rearrange("p (kc t) -> p kc t", kc=NKC),
                in_=out_cc[h][rr * K:(rr + 1) * K, :]
                    .rearrange("(kc p) t -> p kc t", p=128))
            return rkt[:].rearrange("p (kc t) -> p kc t", kc=NKC)

        order = [(h, rr) for h in range(2) for rr in range(NCORES)]
        cur = readback(*order[0])
        vcount = 0
        for oi, (h, rr) in enumerate(order):
            nxt = readback(*order[oi + 1]) if oi + 1 < len(order) else None
            for cl in (0, 1):
                t0 = (rr * LC + h * 2 + cl) * 128
                for v in range(nv):
                    w, v0 = vws[v], voff[v]
                    ph = psh.tile([128, 512], f32, tag="psh")
                    for kc in range(NKC):
                        nc.tensor.matmul(
                            out=ph[:, :w],
                            lhsT=cur[:, kc, cl * 128:(cl + 1) * 128],
                            rhs=wt3[:, kc, v0:v0 + w],
                            start=(kc == 0), stop=(kc == NKC - 1))
                    stg = outp.tile([128, 512], bf16, tag="stg")
                    if vcount % 2 == 0:
                        nc.scalar.copy(stg[:, :w], ph[:, :w])
                    else:
                        nc.vector.tensor_copy(stg[:, :w], ph[:, :w])
                    deng = nc.sync if vcount % 2 == 0 else nc.gpsimd
                    deng.dma_start(out=out_p[t0:t0 + 128, v0:v0 + w],
                                   in_=stg[:, :w])
                    vcount += 1
            cur = nxt

    if not nc.is_finalized():
        nc.finalize()
    return nc


_NC_CACHE = {}


def _get_nc(cfg):
    key = (cfg.vs, cfg.nt)
    if key not in _NC_CACHE:
        _NC_CACHE[key] = _build(cfg)
    return _NC_CACHE[key]


def _make_in_maps(cfg, input_ids, embed, w_inner, w_head, phi, b,
                  alpha_pre, alpha_post, alpha_res):
    import ml_dtypes
    bf = ml_dtypes.bfloat16

    ids = np.asarray(input_ids).astype(np.int64).reshape(-1)     # [NT]
    x = np.asarray(embed).astype(bf)[ids]                        # [NT, K]
    phi_np = np.asarray(phi).astype(bf)
    wit = np.ascontiguousarray(np.asarray(w_inner).astype(bf).T)
    b_np = np.asarray(b, dtype=np.float32).reshape(1, M)
    al = np.array([[np.asarray(alpha_pre).reshape(-1)[0],
                    np.asarray(alpha_post).reshape(-1)[0],
                    np.asarray(alpha_res).reshape(-1)[0]]], dtype=np.float32)
    wh_t = np.asarray(w_head).astype(bf).T                       # [K, vocab]

    in_maps = []
    for i in range(NCORES):
        sl = wh_t[:, i * VS:(i + 1) * VS]
        if sl.shape[1] < VS:
            wvt = np.zeros((K, VS), bf)
            wvt[:, :sl.shape[1]] = sl
        else:
            wvt = np.ascontiguousarray(sl)
        xr = np.ascontiguousarray(x[i * TPC:(i + 1) * TPC])
        in_maps.append(dict(xr=xr, wvt=wvt, wit=wit, phi=phi_np,
                            b=b_np, al=al))
    return in_maps


def _run(cfg, in_maps, trace=False):
    from concourse.bass_utils import run_bass_kernel_spmd
    nc = _get_nc(cfg)
    return run_bass_kernel_spmd(nc, in_maps, list(range(NCORES)), trace=trace)


def _run2(cfg, in_maps, trace=False, trace_cores=None):
    from concourse.bass_utils import run_bass_kernel_spmd
    nc = _get_nc(cfg)
    return run_bass_kernel_spmd(nc, in_maps, list(range(NCORES)), trace=trace,
                                trace_cores=trace_cores)


def kernel(input_ids, embed, w_inner, w_head, phi, b,
           alpha_pre, alpha_post, alpha_res):
    cfg = REAL
    in_maps = _make_in_maps(cfg, input_ids, embed, w_inner, w_head, phi, b,
                            alpha_pre, alpha_post, alpha_res)
    res = _run(cfg, in_maps).results
    out = np.concatenate(
        [np.asarray(res[i]["out"]).astype(np.float32) for i in range(NCORES)],
        axis=1)[:, :VOCAB]
    return np.ascontiguousarray(out.reshape(B, S, VOCAB))


# revision 12
# speedup vs baseline: 1.2671x; 1.2185x over previous
"""Trainium2 Bass kernel for nn_MiniMHCLM (moe_routing).

Strategy (8 NeuronCores, SPMD, fully independent cores — no collectives):
  - embedding gather happens HOST-side (embed[ids] -> bf16, numerically
    identical to the reference's cast); each core is shipped its
    512-token slice (1 MB) instead of the 103 MB table.
  - per-token pipeline (RMS+phi coeffs, Sinkhorn, gather/scatter mixing)
    is DATA-PARALLEL: core r processes tokens [512r, 512r+512).
  - TOKEN-sharded head matmul: each core computes logits for its own
    512 tokens x the FULL vocab, streaming w_head^T (bf16, k-major)
    from HBM in 512-column blocks, triple-buffered; each block is
    reused by all 4 local chunks, so HBM read (103 MB) stays well under
    the matmul time. Host concatenates along tokens.
  - the shared inner linear is computed per-head (g_i = x_i @ w_inner^T)
    from the already-transposed x tiles; f_out = sum_i h_pre[i] * g_i.
  - head matmul bf16 with fp32 PSUM accumulation; PSUM evacuated by
    alternating ACT/DVE copies to bf16, DMA'd out on two queues.
  - output is bf16 on device (halves HBM writes + host download); host
    converts to fp32.
"""

import numpy as np

HC, C, TMAX = 4, 256, 8
RMS_EPS, PRE_EPS, SINK_EPS, POST_MULT = 1e-6, 1e-4, 1e-6, 2.0
VOCAB = 50257
B, S = 2, 2048
K = HC * C            # 1024
M = HC * HC + 2 * HC  # 24
NKC = K // 128        # 8 k-chunks
NCORES = 8
NT = B * S            # 4096 tokens
TPC = NT // NCORES    # 512 tokens per core
LC = TPC // 128       # 4 local 128-token chunks
VW = 512              # head vocab-block width


class _Cfg:
    def __init__(self):
        self.nt = NT
        # head v-blocks over the FULL vocab: 98 x 512 + 81
        self.vws = [VW] * (VOCAB // VW) + ([VOCAB % VW] if VOCAB % VW else [])


REAL = _Cfg()


def _build(cfg: _Cfg):
    from contextlib import ExitStack
    from concourse import bass, bacc, mybir
    import concourse.tile as tile
    from concourse.masks import make_identity

    f32 = mybir.dt.float32
    bf16 = mybir.dt.bfloat16
    AX = mybir.AxisListType
    OP = mybir.AluOpType
    AF = mybir.ActivationFunctionType

    vws = cfg.vws
    nv = len(vws)
    voff = [sum(vws[:i]) for i in range(nv)]

    nc = bacc.Bacc(target_bir_lowering=False, num_devices=NCORES)
    xr_p = nc.declare_dram_parameter("xr", [TPC, K], bf16, False)
    wvt_p = nc.declare_dram_parameter("wvt", [K, VOCAB], bf16, False)
    wit_p = nc.declare_dram_parameter("wit", [C, C], bf16, False)
    phi_p = nc.declare_dram_parameter("phi", [K, M], bf16, False)
    b_p = nc.declare_dram_parameter("b", [1, M], f32, False)
    al_p = nc.declare_dram_parameter("al", [1, 3], f32, False)
    out_p = nc.declare_dram_parameter("out", [TPC, VOCAB], bf16, True)

    with ExitStack() as ctx:
        tc = ctx.enter_context(tile.TileContext(nc))
        const = ctx.enter_context(tc.tile_pool(name="const", bufs=1))
        xbfp = ctx.enter_context(tc.tile_pool(name="xbfp", bufs=1))
        scp = ctx.enter_context(tc.tile_pool(name="scp", bufs=1))
        wkA = ctx.enter_context(tc.tile_pool(name="wkA", bufs=2))
        wkB = ctx.enter_context(tc.tile_pool(name="wkB", bufs=2))
        wkC = ctx.enter_context(tc.tile_pool(name="wkC", bufs=2))
        wk3 = ctx.enter_context(tc.tile_pool(name="wk3", bufs=6))
        wsp = ctx.enter_context(tc.tile_pool(name="wsp", bufs=4))
        outp = ctx.enter_context(tc.tile_pool(name="outp", bufs=12))
        pst = ctx.enter_context(tc.tile_pool(name="pst", bufs=2, space="PSUM"))
        psc = ctx.enter_context(tc.tile_pool(name="psc", bufs=1, space="PSUM"))
        psg = ctx.enter_context(tc.tile_pool(name="psg", bufs=1, space="PSUM"))
        psh = ctx.enter_context(tc.tile_pool(name="psh", bufs=3, space="PSUM"))

        # ---------------- prep ----------------
        # this core's token slice goes FIRST on the sync ring: the whole
        # pipeline hangs off it and its DMAHW completion lane must not
        # queue behind anything slow.
        xr_sb = xbfp.tile([128, LC * K], bf16, tag="xr")
        xr3 = xr_sb[:].rearrange("p (c k) -> p c k", c=LC)
        nc.sync.dma_start(
            out=xr3, in_=xr_p[:, :].rearrange("(c p) k -> p c k", p=128))

        ident = const.tile([128, 128], bf16)
        make_identity(nc, ident[:])

        cst = const.tile([128, 2], f32)
        nc.vector.memset(cst[:, 0:1], 0.0)
        nc.vector.memset(cst[:, 1:2], RMS_EPS)
        zero_b = cst[:, 0:1]
        eps_b = cst[:, 1:2]

        phi_sb = const.tile([128, NKC * M], bf16)
        for kc in range(NKC):
            nc.sync.dma_start(out=phi_sb[:, kc * M:(kc + 1) * M],
                              in_=phi_p[kc * 128:(kc + 1) * 128, :])
        b_bc = const.tile([128, M], f32)
        nc.sync.dma_start(out=b_bc[:], in_=b_p[0:1, :].to_broadcast([128, M]))
        al_bc = const.tile([128, 3], f32)
        nc.sync.dma_start(out=al_bc[:], in_=al_p[0:1, :].to_broadcast([128, 3]))

        # w_inner^T (host-pretransposed bf16, k-major): one strided DMA
        w_iT = const.tile([128, 2 * C], bf16)
        nc.sync.dma_start(
            out=w_iT[:].rearrange("p (kc o) -> p kc o", kc=2),
            in_=wit_p[:, :].rearrange("(kc p) o -> p kc o", p=128))

        # ---------------- local pipeline (LC chunks) ----------------
        # m coefficients stored (i outer, o inner): the mix stage reads a
        # contiguous [128,4] o-column per source head i.
        m_all = scp.tile([128, LC * 16], f32, tag="m_all")
        hpre = scp.tile([128, LC * 4], f32, tag="hpre")
        hpost2 = scp.tile([128, LC * 4], f32, tag="hpost2")
        scl = scp.tile([128, LC], f32, tag="scl")
        sqs = scp.tile([128, LC], f32, tag="sqs")
        lg = scp.tile([128, LC * M], f32, tag="lg")
        # all LC chunks' x_merge^T, k-major: [128, kc, tok(LC*128)]
        xmT_all = scp.tile([128, NKC * TPC], bf16, tag="xmT_all")
        xmT3 = xmT_all[:].rearrange("p (kc t) -> p kc t", kc=NKC)

        g_sbs = {}

        def pass_a(c):
            xb = xr3[:, c, :]
            dump = wkA.tile([128, K], bf16, tag="dump")
            nc.scalar.activation(out=dump[:], in_=xb, func=AF.Square,
                                 bias=zero_b, accum_out=sqs[:, c:c + 1])
            ptx = pst.tile([128, 1024], bf16, tag="pst")
            for kc in range(NKC):
                nc.tensor.transpose(
                    out=ptx[:, kc * 128:(kc + 1) * 128],
                    in_=xb[:, kc * 128:(kc + 1) * 128], identity=ident[:])
            xT = wkA.tile([128, 1024], bf16, tag="xT")
            nc.vector.tensor_copy(xT[:], ptx[:])
            pc = psc.tile([128, C], f32, tag="psc")
            for kc in range(NKC):
                nc.tensor.matmul(
                    out=pc[:, :M],
                    lhsT=xT[:, kc * 128:(kc + 1) * 128],
                    rhs=phi_sb[:, kc * M:(kc + 1) * M],
                    start=(kc == 0), stop=(kc == NKC - 1))
            nc.vector.tensor_copy(lg[:, c * M:(c + 1) * M], pc[:, :M])
            # per-head inner linear: g[t, i*C+o] = sum_k x_i[t,k] w_inner[o,k]
            pg = psg.tile([128, K], f32, tag="psg")
            for kc in range(NKC):
                i = kc // 2
                nc.tensor.matmul(
                    out=pg[:, i * C:(i + 1) * C],
                    lhsT=xT[:, kc * 128:(kc + 1) * 128],
                    rhs=w_iT[:, (kc % 2) * C:(kc % 2 + 1) * C],
                    start=(kc % 2 == 0), stop=(kc % 2 == 1))
            g_sb = wkA.tile([128, K], bf16, tag="g", name=f"g{c}")
            nc.scalar.copy(g_sb[:], pg[:])
            g_sbs[c] = g_sb

        def pass_post():
            g_sc = LC
            nc.scalar.activation(out=scl[:, :g_sc], in_=sqs[:, :g_sc],
                                 func=AF.Sqrt, scale=1.0 / K, bias=eps_b)
            nc.vector.reciprocal(scl[:, :g_sc], scl[:, :g_sc])
            lgv = lg[:, :g_sc * M].rearrange("p (c m) -> p c m", m=M)
            for c in range(g_sc):
                nc.vector.tensor_scalar_mul(
                    lg[:, c * M:(c + 1) * M], lg[:, c * M:(c + 1) * M],
                    scl[:, c:c + 1])
            nc.vector.tensor_tensor(
                out=lgv, in0=lgv,
                in1=b_bc[:][:, None, :].to_broadcast([128, g_sc, M]),
                op=OP.add)
            nc.scalar.activation(
                out=hpre[:, :g_sc * 4].rearrange("p (c f) -> p c f", f=4),
                in_=lgv[:, :, 0:4], func=AF.Sigmoid, bias=zero_b,
                scale=al_bc[:, 0:1])
            nc.vector.tensor_scalar_add(hpre[:, :g_sc * 4],
                                        hpre[:, :g_sc * 4], PRE_EPS)
            nc.scalar.activation(
                out=hpost2[:, :g_sc * 4].rearrange("p (c f) -> p c f", f=4),
                in_=lgv[:, :, 4:8], func=AF.Sigmoid, bias=zero_b,
                scale=al_bc[:, 1:2])
            nc.vector.tensor_scalar_mul(hpost2[:, :g_sc * 4],
                                        hpost2[:, :g_sc * 4], POST_MULT)
            # exp(a_res * res): reference res is [o][i]; store transposed
            # (i outer, o inner) via a strided output view.
            m4t = m_all[:, :g_sc * 16].rearrange(
                "p (c i o) -> p c o i", i=4, o=4)
            nc.scalar.activation(
                out=m4t,
                in_=lgv[:, :, 8:24].rearrange("p c (o i) -> p c o i", o=4),
                func=AF.Exp, bias=zero_b, scale=al_bc[:, 2:3])

            # batched Sinkhorn on (i outer, o inner) storage
            ma = m_all[:, :g_sc * 16]
            mio = ma.rearrange("p (c i o) -> p c i o", i=4, o=4)
            moi = ma.rearrange("p (c i o) -> p c o i", i=4, o=4)
            for _ in range(TMAX):
                # row normalize: per (c, o) sum over i
                rs = wkB.tile([128, LC * 4], f32, tag="rs")
                rs3 = rs[:, :g_sc * 4].rearrange("p (c o) -> p c o", o=4)
                nc.vector.tensor_reduce(rs3, moi, axis=AX.X, op=OP.add)
                nc.vector.reciprocal(rs[:, :g_sc * 4], rs[:, :g_sc * 4])
                nc.vector.tensor_tensor(
                    out=moi, in0=moi,
                    in1=rs3[:, :, :, None].to_broadcast([128, g_sc, 4, 4]),
                    op=OP.mult)
                # col normalize: per (c, i) sum over o
                cs = wkB.tile([128, LC * 4], f32, tag="cs")
                cs3 = cs[:, :g_sc * 4].rearrange("p (c i) -> p c i", i=4)
                nc.vector.tensor_reduce(cs3, mio, axis=AX.X, op=OP.add)
                nc.vector.reciprocal(cs[:, :g_sc * 4], cs[:, :g_sc * 4])
                nc.vector.tensor_tensor(
                    out=mio, in0=mio,
                    in1=cs3[:, :, :, None].to_broadcast([128, g_sc, 4, 4]),
                    op=OP.mult)

        def pass_mix(c):
            xb = xr3[:, c, :]
            # f_out = sum_i h_pre[i] * g_i  (g precomputed in pass_a);
            # doesn't need m_all, so the scheduler can hoist it over the
            # Sinkhorn iterations.
            g_sb = g_sbs.pop(c)
            fo = wkC.tile([128, C], f32, tag="fo")
            nc.vector.tensor_scalar_mul(
                fo[:], g_sb[:, 0:C], hpre[:, c * 4:c * 4 + 1])
            for i in range(1, HC):
                tmp = wk3.tile([128, C], f32, tag="ftmp")
                nc.vector.tensor_scalar_mul(
                    tmp[:], g_sb[:, i * C:(i + 1) * C],
                    hpre[:, c * 4 + i:c * 4 + i + 1])
                eng = nc.vector if i % 2 else nc.gpsimd
                eng.tensor_add(fo[:], fo[:], tmp[:])

            # x_merge[o*C+c2] = sum_i m[i-outer][o] x_i[c2] + h_post2[o] fo
            xmg = wkC.tile([128, K], bf16, tag="xmg")
            for o in range(HC):
                seg = xmg[:, o * C:(o + 1) * C]
                nc.vector.tensor_scalar_mul(
                    seg, xb[:, 0:C], m_all[:, c * 16 + o:c * 16 + o + 1])
                for i in range(1, HC):
                    tmp = wk3.tile([128, C], bf16, tag="tmp")
                    nc.vector.tensor_scalar_mul(
                        tmp[:], xb[:, i * C:(i + 1) * C],
                        m_all[:, c * 16 + i * 4 + o:c * 16 + i * 4 + o + 1])
                    eng = nc.vector if i % 2 else nc.gpsimd
                    eng.tensor_add(seg, seg, tmp[:])
                tmp = wk3.tile([128, C], bf16, tag="tmp")
                nc.vector.tensor_scalar_mul(
                    tmp[:], fo[:], hpost2[:, c * 4 + o:c * 4 + o + 1])
                nc.gpsimd.tensor_add(seg, seg, tmp[:])

            ptm = pst.tile([128, 1024], bf16, tag="pst")
            for kc in range(NKC):
                nc.tensor.transpose(
                    out=ptm[:, kc * 128:(kc + 1) * 128],
                    in_=xmg[:, kc * 128:(kc + 1) * 128], identity=ident[:])
            nc.scalar.copy(
                xmT3[:, :, c * 128:(c + 1) * 128],
                ptm[:].rearrange("p (kc t) -> p kc t", kc=NKC))

        for c in range(LC):
            pass_a(c)
        pass_post()
        for c in range(LC):
            pass_mix(c)

        # ---------------- token-sharded head over full vocab ----------------
        # Stream w_head^T in 512-column blocks on the scalar HWDGE ring,
        # triple-buffered; each block serves all LC chunks.
        def wfetch(v):
            w = vws[v]
            wtile = wsp.tile([128, NKC * VW], bf16, tag="ws", name=f"ws{v}")
            nc.scalar.dma_start(
                out=wtile[:].rearrange("p (kc c) -> p kc c", kc=NKC)[:, :, :w],
                in_=wvt_p[:, voff[v]:voff[v] + w]
                    .rearrange("(kc p) c -> p kc c", p=128))
            return wtile

        wtiles = {v: wfetch(v) for v in range(min(4, nv))}
        vcount = 0
        for v in range(nv):
            w, v0 = vws[v], voff[v]
            if v + 4 < nv:
                wtiles[v + 4] = wfetch(v + 4)
            wtile = wtiles.pop(v)
            wv = wtile[:].rearrange("p (kc c) -> p kc c", kc=NKC)
            for cl in range(LC):
                ph = psh.tile([128, 512], f32, tag="psh")
                for kc in range(NKC):
                    nc.tensor.matmul(
                        out=ph[:, :w],
                        lhsT=xmT3[:, kc, cl * 128:(cl + 1) * 128],
                        rhs=wv[:, kc, :w],
                        start=(kc == 0), stop=(kc == NKC - 1))
                stg = outp.tile([128, 512], bf16, tag="stg")
                if vcount % 2 == 0:
                    nc.scalar.copy(stg[:, :w], ph[:, :w])
                else:
                    nc.vector.tensor_copy(stg[:, :w], ph[:, :w])
                deng = nc.sync if vcount % 2 == 0 else nc.gpsimd
                deng.dma_start(
                    out=out_p[cl * 128:(cl + 1) * 128, v0:v0 + w],
                    in_=stg[:, :w])
                vcount += 1

    if not nc.is_finalized():
        nc.finalize()
    return nc


_NC_CACHE = {}


def _get_nc(cfg):
    key = (cfg.nt, len(cfg.vws))
    if key not in _NC_CACHE:
        _NC_CACHE[key] = _build(cfg)
    return _NC_CACHE[key]


def _make_in_maps(cfg, input_ids, embed, w_inner, w_head, phi, b,
                  alpha_pre, alpha_post, alpha_res):
    import ml_dtypes
    bf = ml_dtypes.bfloat16

    ids = np.asarray(input_ids).astype(np.int64).reshape(-1)     # [NT]
    x = np.asarray(embed).astype(bf)[ids]                        # [NT, K]
    phi_np = np.asarray(phi).astype(bf)
    wit = np.ascontiguousarray(np.asarray(w_inner).astype(bf).T)
    b_np = np.asarray(b, dtype=np.float32).reshape(1, M)
    al = np.array([[np.asarray(alpha_pre).reshape(-1)[0],
                    np.asarray(alpha_post).reshape(-1)[0],
                    np.asarray(alpha_res).reshape(-1)[0]]], dtype=np.float32)
    wvt = np.ascontiguousarray(np.asarray(w_head).astype(bf).T)  # [K, vocab]

    in_maps = []
    for i in range(NCORES):
        xr = np.ascontiguousarray(x[i * TPC:(i + 1) * TPC])
        in_maps.append(dict(xr=xr, wvt=wvt, wit=wit, phi=phi_np,
                            b=b_np, al=al))
    return in_maps


def _run(cfg, in_maps, trace=False):
    from concourse.bass_utils import run_bass_kernel_spmd
    nc = _get_nc(cfg)
    return run_bass_kernel_spmd(nc, in_maps, list(range(NCORES)), trace=trace)


def _run2(cfg, in_maps, trace=False, trace_cores=None):
    from concourse.bass_utils import run_bass_kernel_spmd
    nc = _get_nc(cfg)
    return run_bass_kernel_spmd(nc, in_maps, list(range(NCORES)), trace=trace,
                                trace_cores=trace_cores)


def kernel(input_ids, embed, w_inner, w_head, phi, b,
           alpha_pre, alpha_post, alpha_res):
    cfg = REAL
    in_maps = _make_in_maps(cfg, input_ids, embed, w_inner, w_head, phi, b,
                            alpha_pre, alpha_post, alpha_res)
    res = _run(cfg, in_maps).results
    out = np.concatenate(
        [np.asarray(res[i]["out"]).astype(np.float32) for i in range(NCORES)],
        axis=0)
    return np.ascontiguousarray(out.reshape(B, S, VOCAB))


# revision 13
# speedup vs baseline: 1.3217x; 1.0431x over previous
"""Trainium2 Bass kernel for nn_MiniMHCLM (moe_routing).

Strategy (8 NeuronCores, SPMD, fully independent cores — no collectives):
  - embedding gather happens HOST-side (embed[ids] -> bf16, numerically
    identical to the reference's cast); each core is shipped its
    512-token slice (1 MB) instead of the 103 MB table.
  - per-token pipeline (RMS+phi coeffs, Sinkhorn, gather/scatter mixing)
    is DATA-PARALLEL: core r processes tokens [512r, 512r+512).
  - TOKEN-sharded head matmul: each core computes logits for its own
    512 tokens x the FULL vocab, streaming w_head^T (bf16, k-major)
    from HBM in 512-column blocks, triple-buffered; each block is
    reused by all 4 local chunks, so HBM read (103 MB) stays well under
    the matmul time. Host concatenates along tokens.
  - the shared inner linear is computed per-head (g_i = x_i @ w_inner^T)
    from the already-transposed x tiles; f_out = sum_i h_pre[i] * g_i.
  - head matmul bf16 with fp32 PSUM accumulation; PSUM evacuated by
    alternating ACT/DVE copies to bf16, DMA'd out on two queues.
  - output is bf16 on device (halves HBM writes + host download); host
    converts to fp32.
"""

import numpy as np

HC, C, TMAX = 4, 256, 8
RMS_EPS, PRE_EPS, SINK_EPS, POST_MULT = 1e-6, 1e-4, 1e-6, 2.0
VOCAB = 50257
B, S = 2, 2048
K = HC * C            # 1024
M = HC * HC + 2 * HC  # 24
NKC = K // 128        # 8 k-chunks
NCORES = 8
NT = B * S            # 4096 tokens
TPC = NT // NCORES    # 512 tokens per core
LC = TPC // 128       # 4 local 128-token chunks
VW = 512              # head vocab-block width


class _Cfg:
    def __init__(self):
        self.nt = NT
        # head v-blocks over the FULL vocab: 98 x 512 + 81
        self.vws = [VW] * (VOCAB // VW) + ([VOCAB % VW] if VOCAB % VW else [])


REAL = _Cfg()


def _build(cfg: _Cfg):
    from contextlib import ExitStack
    from concourse import bass, bacc, mybir
    import concourse.tile as tile
    from concourse.masks import make_identity

    f32 = mybir.dt.float32
    bf16 = mybir.dt.bfloat16
    AX = mybir.AxisListType
    OP = mybir.AluOpType
    AF = mybir.ActivationFunctionType

    vws = cfg.vws
    nv = len(vws)
    voff = [sum(vws[:i]) for i in range(nv)]

    nc = bacc.Bacc(target_bir_lowering=False, num_devices=NCORES)
    xr_p = nc.declare_dram_parameter("xr", [TPC, K], bf16, False)
    wvt_p = nc.declare_dram_parameter("wvt", [K, VOCAB], bf16, False)
    wit_p = nc.declare_dram_parameter("wit", [C, C], bf16, False)
    phi_p = nc.declare_dram_parameter("phi", [K, M], bf16, False)
    b_p = nc.declare_dram_parameter("b", [1, M], f32, False)
    al_p = nc.declare_dram_parameter("al", [1, 3], f32, False)
    out_p = nc.declare_dram_parameter("out", [TPC, VOCAB], bf16, True)

    with ExitStack() as ctx:
        tc = ctx.enter_context(tile.TileContext(nc))
        const = ctx.enter_context(tc.tile_pool(name="const", bufs=1))
        xbfp = ctx.enter_context(tc.tile_pool(name="xbfp", bufs=1))
        scp = ctx.enter_context(tc.tile_pool(name="scp", bufs=1))
        wkA = ctx.enter_context(tc.tile_pool(name="wkA", bufs=2))
        wkB = ctx.enter_context(tc.tile_pool(name="wkB", bufs=2))
        wkC = ctx.enter_context(tc.tile_pool(name="wkC", bufs=2))
        wk3 = ctx.enter_context(tc.tile_pool(name="wk3", bufs=6))
        wsp = ctx.enter_context(tc.tile_pool(name="wsp", bufs=4))
        outp = ctx.enter_context(tc.tile_pool(name="outp", bufs=12))
        pst = ctx.enter_context(tc.tile_pool(name="pst", bufs=2, space="PSUM"))
        psc = ctx.enter_context(tc.tile_pool(name="psc", bufs=1, space="PSUM"))
        psg = ctx.enter_context(tc.tile_pool(name="psg", bufs=1, space="PSUM"))
        psh = ctx.enter_context(tc.tile_pool(name="psh", bufs=3, space="PSUM"))

        # ---------------- prep ----------------
        # this core's token slice goes FIRST on the sync ring: the whole
        # pipeline hangs off it and its DMAHW completion lane must not
        # queue behind anything slow.
        xr_sb = xbfp.tile([128, LC * K], bf16, tag="xr")
        xr3 = xr_sb[:].rearrange("p (c k) -> p c k", c=LC)
        nc.sync.dma_start(
            out=xr3, in_=xr_p[:, :].rearrange("(c p) k -> p c k", p=128))

        ident = const.tile([128, 128], bf16)
        make_identity(nc, ident[:])

        cst = const.tile([128, 2], f32)
        nc.vector.memset(cst[:, 0:1], 0.0)
        nc.vector.memset(cst[:, 1:2], RMS_EPS)
        zero_b = cst[:, 0:1]
        eps_b = cst[:, 1:2]

        phi_sb = const.tile([128, NKC * M], bf16)
        for kc in range(NKC):
            nc.sync.dma_start(out=phi_sb[:, kc * M:(kc + 1) * M],
                              in_=phi_p[kc * 128:(kc + 1) * 128, :])
        b_bc = const.tile([128, M], f32)
        nc.sync.dma_start(out=b_bc[:], in_=b_p[0:1, :].to_broadcast([128, M]))
        al_bc = const.tile([128, 3], f32)
        nc.sync.dma_start(out=al_bc[:], in_=al_p[0:1, :].to_broadcast([128, 3]))

        # w_inner^T (host-pretransposed bf16, k-major): one strided DMA
        w_iT = const.tile([128, 2 * C], bf16)
        nc.sync.dma_start(
            out=w_iT[:].rearrange("p (kc o) -> p kc o", kc=2),
            in_=wit_p[:, :].rearrange("(kc p) o -> p kc o", p=128))

        # ---------------- local pipeline (LC chunks) ----------------
        # m coefficients stored (i outer, o inner): the mix stage reads a
        # contiguous [128,4] o-column per source head i.
        m_all = scp.tile([128, LC * 16], f32, tag="m_all")
        hpre = scp.tile([128, LC * 4], f32, tag="hpre")
        hpost2 = scp.tile([128, LC * 4], f32, tag="hpost2")
        scl = scp.tile([128, LC], f32, tag="scl")
        sqs = scp.tile([128, LC], f32, tag="sqs")
        lg = scp.tile([128, LC * M], f32, tag="lg")
        # all LC chunks' x_merge^T, k-major: [128, kc, tok(LC*128)]
        xmT_all = scp.tile([128, NKC * TPC], bf16, tag="xmT_all")
        xmT3 = xmT_all[:].rearrange("p (kc t) -> p kc t", kc=NKC)

        g_sbs = {}

        def pass_a(c):
            xb = xr3[:, c, :]
            dump = wkA.tile([128, K], bf16, tag="dump")
            nc.scalar.activation(out=dump[:], in_=xb, func=AF.Square,
                                 bias=zero_b, accum_out=sqs[:, c:c + 1])
            ptx = pst.tile([128, 1024], bf16, tag="pst")
            for kc in range(NKC):
                nc.tensor.transpose(
                    out=ptx[:, kc * 128:(kc + 1) * 128],
                    in_=xb[:, kc * 128:(kc + 1) * 128], identity=ident[:])
            xT = wkA.tile([128, 1024], bf16, tag="xT")
            nc.vector.tensor_copy(xT[:], ptx[:])
            pc = psc.tile([128, C], f32, tag="psc")
            for kc in range(NKC):
                nc.tensor.matmul(
                    out=pc[:, :M],
                    lhsT=xT[:, kc * 128:(kc + 1) * 128],
                    rhs=phi_sb[:, kc * M:(kc + 1) * M],
                    start=(kc == 0), stop=(kc == NKC - 1))
            nc.vector.tensor_copy(lg[:, c * M:(c + 1) * M], pc[:, :M])
            # per-head inner linear: g[t, i*C+o] = sum_k x_i[t,k] w_inner[o,k]
            pg = psg.tile([128, K], f32, tag="psg")
            for kc in range(NKC):
                i = kc // 2
                nc.tensor.matmul(
                    out=pg[:, i * C:(i + 1) * C],
                    lhsT=xT[:, kc * 128:(kc + 1) * 128],
                    rhs=w_iT[:, (kc % 2) * C:(kc % 2 + 1) * C],
                    start=(kc % 2 == 0), stop=(kc % 2 == 1))
            g_sb = wkA.tile([128, K], bf16, tag="g", name=f"g{c}")
            nc.scalar.copy(g_sb[:], pg[:])
            g_sbs[c] = g_sb

        def pass_post():
            g_sc = LC
            nc.scalar.activation(out=scl[:, :g_sc], in_=sqs[:, :g_sc],
                                 func=AF.Sqrt, scale=1.0 / K, bias=eps_b)
            nc.vector.reciprocal(scl[:, :g_sc], scl[:, :g_sc])
            lgv = lg[:, :g_sc * M].rearrange("p (c m) -> p c m", m=M)
            for c in range(g_sc):
                nc.vector.tensor_scalar_mul(
                    lg[:, c * M:(c + 1) * M], lg[:, c * M:(c + 1) * M],
                    scl[:, c:c + 1])
            nc.vector.tensor_tensor(
                out=lgv, in0=lgv,
                in1=b_bc[:][:, None, :].to_broadcast([128, g_sc, M]),
                op=OP.add)
            nc.scalar.activation(
                out=hpre[:, :g_sc * 4].rearrange("p (c f) -> p c f", f=4),
                in_=lgv[:, :, 0:4], func=AF.Sigmoid, bias=zero_b,
                scale=al_bc[:, 0:1])
            nc.vector.tensor_scalar_add(hpre[:, :g_sc * 4],
                                        hpre[:, :g_sc * 4], PRE_EPS)
            nc.scalar.activation(
                out=hpost2[:, :g_sc * 4].rearrange("p (c f) -> p c f", f=4),
                in_=lgv[:, :, 4:8], func=AF.Sigmoid, bias=zero_b,
                scale=al_bc[:, 1:2])
            nc.vector.tensor_scalar_mul(hpost2[:, :g_sc * 4],
                                        hpost2[:, :g_sc * 4], POST_MULT)
            # exp(a_res * res): reference res is [o][i]; store transposed
            # (i outer, o inner) via a strided output view.
            m4t = m_all[:, :g_sc * 16].rearrange(
                "p (c i o) -> p c o i", i=4, o=4)
            nc.scalar.activation(
                out=m4t,
                in_=lgv[:, :, 8:24].rearrange("p c (o i) -> p c o i", o=4),
                func=AF.Exp, bias=zero_b, scale=al_bc[:, 2:3])

            # batched Sinkhorn on (i outer, o inner) storage
            ma = m_all[:, :g_sc * 16]
            mio = ma.rearrange("p (c i o) -> p c i o", i=4, o=4)
            moi = ma.rearrange("p (c i o) -> p c o i", i=4, o=4)
            for _ in range(TMAX):
                # row normalize: per (c, o) sum over i
                rs = wkB.tile([128, LC * 4], f32, tag="rs")
                rs3 = rs[:, :g_sc * 4].rearrange("p (c o) -> p c o", o=4)
                nc.vector.tensor_reduce(rs3, moi, axis=AX.X, op=OP.add)
                nc.vector.reciprocal(rs[:, :g_sc * 4], rs[:, :g_sc * 4])
                nc.vector.tensor_tensor(
                    out=moi, in0=moi,
                    in1=rs3[:, :, :, None].to_broadcast([128, g_sc, 4, 4]),
                    op=OP.mult)
                # col normalize: per (c, i) sum over o
                cs = wkB.tile([128, LC * 4], f32, tag="cs")
                cs3 = cs[:, :g_sc * 4].rearrange("p (c i) -> p c i", i=4)
                nc.vector.tensor_reduce(cs3, mio, axis=AX.X, op=OP.add)
                nc.vector.reciprocal(cs[:, :g_sc * 4], cs[:, :g_sc * 4])
                nc.vector.tensor_tensor(
                    out=mio, in0=mio,
                    in1=cs3[:, :, :, None].to_broadcast([128, g_sc, 4, 4]),
                    op=OP.mult)

        def pass_mix(c):
            xb = xr3[:, c, :]
            # f_out = sum_i h_pre[i] * g_i  (g precomputed in pass_a);
            # doesn't need m_all, so the scheduler can hoist it over the
            # Sinkhorn iterations.
            g_sb = g_sbs.pop(c)
            fo = wkC.tile([128, C], f32, tag="fo")
            nc.vector.tensor_scalar_mul(
                fo[:], g_sb[:, 0:C], hpre[:, c * 4:c * 4 + 1])
            for i in range(1, HC):
                tmp = wk3.tile([128, C], f32, tag="ftmp")
                nc.vector.tensor_scalar_mul(
                    tmp[:], g_sb[:, i * C:(i + 1) * C],
                    hpre[:, c * 4 + i:c * 4 + i + 1])
                eng = nc.vector if i % 2 else nc.gpsimd
                eng.tensor_add(fo[:], fo[:], tmp[:])

            # x_merge[o*C+c2] = sum_i m[i-outer][o] x_i[c2] + h_post2[o] fo
            xmg = wkC.tile([128, K], bf16, tag="xmg")
            for o in range(HC):
                seg = xmg[:, o * C:(o + 1) * C]
                nc.vector.tensor_scalar_mul(
                    seg, xb[:, 0:C], m_all[:, c * 16 + o:c * 16 + o + 1])
                for i in range(1, HC):
                    tmp = wk3.tile([128, C], bf16, tag="tmp")
                    nc.vector.tensor_scalar_mul(
                        tmp[:], xb[:, i * C:(i + 1) * C],
                        m_all[:, c * 16 + i * 4 + o:c * 16 + i * 4 + o + 1])
                    eng = nc.vector if i % 2 else nc.gpsimd
                    eng.tensor_add(seg, seg, tmp[:])
                tmp = wk3.tile([128, C], bf16, tag="tmp")
                nc.vector.tensor_scalar_mul(
                    tmp[:], fo[:], hpost2[:, c * 4 + o:c * 4 + o + 1])
                nc.gpsimd.tensor_add(seg, seg, tmp[:])

            ptm = pst.tile([128, 1024], bf16, tag="pst")
            for kc in range(NKC):
                nc.tensor.transpose(
                    out=ptm[:, kc * 128:(kc + 1) * 128],
                    in_=xmg[:, kc * 128:(kc + 1) * 128], identity=ident[:])
            nc.scalar.copy(
                xmT3[:, :, c * 128:(c + 1) * 128],
                ptm[:].rearrange("p (kc t) -> p kc t", kc=NKC))

        for c in range(LC):
            pass_a(c)
        pass_post()
        for c in range(LC):
            pass_mix(c)

        # ---------------- token-sharded head over full vocab ----------------
        # Stream w_head^T in 1024-column fetch blocks (2 KB descriptors) on
        # the scalar HWDGE ring, triple-buffered; each block serves all LC
        # chunks. 50257 = 81 + 49*1024 — the ragged 81-col block goes FIRST
        # so the kernel doesn't end on a slow strided fetch.
        FW = 2 * VW
        fws = [VOCAB % FW] + [FW] * (VOCAB // FW)
        foff = [sum(fws[:i]) for i in range(len(fws))]

        def wfetch(j):
            fw = fws[j]
            wtile = wsp.tile([128, NKC * FW], bf16, tag="ws", name=f"ws{j}")
            nc.scalar.dma_start(
                out=wtile[:].rearrange("p (kc c) -> p kc c",
                                       kc=NKC)[:, :, :fw],
                in_=wvt_p[:, foff[j]:foff[j] + fw]
                    .rearrange("(kc p) c -> p kc c", p=128))
            return wtile

        nf = len(fws)
        wtiles = {j: wfetch(j) for j in range(min(3, nf))}
        vcount = 0
        for j in range(nf):
            if j + 3 < nf:
                wtiles[j + 3] = wfetch(j + 3)
            wtile = wtiles.pop(j)
            wv = wtile[:].rearrange("p (kc c) -> p kc c", kc=NKC)
            subs = [fws[j]] if fws[j] <= VW else [VW, VW]
            for si, w in enumerate(subs):
                s0 = si * VW
                for cl in range(LC):
                    ph = psh.tile([128, 512], f32, tag="psh")
                    for kc in range(NKC):
                        nc.tensor.matmul(
                            out=ph[:, :w],
                            lhsT=xmT3[:, kc, cl * 128:(cl + 1) * 128],
                            rhs=wv[:, kc, s0:s0 + w],
                            start=(kc == 0), stop=(kc == NKC - 1))
                    stg = outp.tile([128, 512], bf16, tag="stg")
                    if vcount % 2 == 0:
                        nc.scalar.copy(stg[:, :w], ph[:, :w])
                    else:
                        nc.vector.tensor_copy(stg[:, :w], ph[:, :w])
                    deng = nc.sync if vcount % 2 == 0 else nc.gpsimd
                    v0 = foff[j] + s0
                    deng.dma_start(
                        out=out_p[cl * 128:(cl + 1) * 128, v0:v0 + w],
                        in_=stg[:, :w])
                    vcount += 1

    if not nc.is_finalized():
        nc.finalize()
    return nc


_NC_CACHE = {}


def _get_nc(cfg):
    key = (cfg.nt, len(cfg.vws))
    if key not in _NC_CACHE:
        _NC_CACHE[key] = _build(cfg)
    return _NC_CACHE[key]


def _make_in_maps(cfg, input_ids, embed, w_inner, w_head, phi, b,
                  alpha_pre, alpha_post, alpha_res):
    import ml_dtypes
    bf = ml_dtypes.bfloat16

    ids = np.asarray(input_ids).astype(np.int64).reshape(-1)     # [NT]
    x = np.asarray(embed).astype(bf)[ids]                        # [NT, K]
    phi_np = np.asarray(phi).astype(bf)
    wit = np.ascontiguousarray(np.asarray(w_inner).astype(bf).T)
    b_np = np.asarray(b, dtype=np.float32).reshape(1, M)
    al = np.array([[np.asarray(alpha_pre).reshape(-1)[0],
                    np.asarray(alpha_post).reshape(-1)[0],
                    np.asarray(alpha_res).reshape(-1)[0]]], dtype=np.float32)
    wvt = np.ascontiguousarray(np.asarray(w_head).astype(bf).T)  # [K, vocab]

    in_maps = []
    for i in range(NCORES):
        xr = np.ascontiguousarray(x[i * TPC:(i + 1) * TPC])
        in_maps.append(dict(xr=xr, wvt=wvt, wit=wit, phi=phi_np,
                            b=b_np, al=al))
    return in_maps


def _run(cfg, in_maps, trace=False):
    from concourse.bass_utils import run_bass_kernel_spmd
    nc = _get_nc(cfg)
    return run_bass_kernel_spmd(nc, in_maps, list(range(NCORES)), trace=trace)


def _run2(cfg, in_maps, trace=False, trace_cores=None):
    from concourse.bass_utils import run_bass_kernel_spmd
    nc = _get_nc(cfg)
    return run_bass_kernel_spmd(nc, in_maps, list(range(NCORES)), trace=trace,
                                trace_cores=trace_cores)


def kernel(input_ids, embed, w_inner, w_head, phi, b,
           alpha_pre, alpha_post, alpha_res):
    cfg = REAL
    in_maps = _make_in_maps(cfg, input_ids, embed, w_inner, w_head, phi, b,
                            alpha_pre, alpha_post, alpha_res)
    res = _run(cfg, in_maps).results
    out = np.concatenate(
        [np.asarray(res[i]["out"]).astype(np.float32) for i in range(NCORES)],
        axis=0)
    return np.ascontiguousarray(out.reshape(B, S, VOCAB))
